# revision 1
# baseline (speedup 1.0000x reference)
"""Multi-head attention (B=2, T=4096, D=512, H=8) on 8 Trainium2 cores.

Sharding: core i handles batch b=i//4, query rows q0=(i%4)*1024 .. q0+1024,
all 8 heads (full K/V of its batch computed on-core; no collectives).
Host pre-transposes x and weights so every DMA is contiguous, and rolls
x along T per core so each core's query block sits at columns 0:1024
(keys become a permutation of T, which attention is invariant to).

All matmuls run in float32r (TF32-like single-pass PE mode, ~1.5e-4 rel
err measured on HW). Softmax skips the max-subtraction (scores are
~N(0, 0.33); exp cannot overflow) and the row-sum comes free from a ones
column appended to V in the attn@V matmul (output partition 64).
"""
import sys
sys.path.insert(0, "/opt/trn_rl_repo")

import numpy as np
import concourse.bacc as bacc
import concourse.mybir as mybir
import concourse.tile as tile
from concourse.bass_utils import run_bass_kernel_spmd

F32 = mybir.dt.float32
F32R = mybir.dt.float32r
AF = mybir.ActivationFunctionType
MULT = mybir.AluOpType.mult

B, T, C = 2, 4096, 512
H, DK = 8, 64
TQ = 1024          # queries per core
NP = 4             # head pairs
KT = T // 128      # 32 k-tiles
CT = C // 128      # 4 contraction tiles

_cache = {}


def _build():
    nc = bacc.Bacc("TRN2")
    xbT = nc.declare_dram_parameter("xbT", [C, T], F32R, isOutput=False)
    wqT = nc.declare_dram_parameter("wqT", [C, C], F32R, isOutput=False)
    wkT = nc.declare_dram_parameter("wkT", [C, C], F32R, isOutput=False)
    wvT = nc.declare_dram_parameter("wvT", [C, C], F32R, isOutput=False)
    woT = nc.declare_dram_parameter("woT", [C, C], F32R, isOutput=False)
    # bias[:, 0] = bq/8, bias[:, 1] = bk, bias[:, 2] = bv  (col-block per pair)
    bias = nc.declare_dram_parameter("bias", [128, 3, NP], F32, isOutput=False)
    bo = nc.declare_dram_parameter("bo", [1, C], F32R, isOutput=False)
    # ind rows: 0 = head0 mask (1s in 0:64), 1 = head1 mask, 2 = all ones
    ind = nc.declare_dram_parameter("ind", [3, 128], F32R, isOutput=False)
    ones = nc.declare_dram_parameter("ones", [128, KT * 4], F32R, isOutput=False)
    out = nc.declare_dram_parameter("out", [TQ, C], F32, isOutput=True)

    with tile.TileContext(nc) as tc:
        attn_bufs, kt_bufs, big_bufs = 4, 2, 3
        use_prj, av_single = False, True
        with (
            tc.tile_pool(name="big", bufs=1) as bpool,
            tc.tile_pool(name="const", bufs=1) as cpool,
            tc.tile_pool(name="work", bufs=2) as wpool,
            tc.tile_pool(name="ktp", bufs=kt_bufs) as ktpool,
            tc.tile_pool(name="attnp", bufs=attn_bufs) as apool,
            tc.tile_pool(name="ps", bufs=big_bufs, space="PSUM") as ps,
            tc.tile_pool(name="prj", bufs=1, space="PSUM") as _psprj,
            tc.tile_pool(name="psav", bufs=1, space="PSUM") as psav,
        ):
            psprj = _psprj if use_prj else ps
            prjtag = "proj" if use_prj else "big"
            # ---- resident tensors ----
            xT = bpool.tile([128, CT, T], F32R, tag="xT")          # 64KB/part
            for ct in range(CT):
                for tch in range(4):
                    nc.sync.dma_start(
                        xT[:, ct, tch * 1024:(tch + 1) * 1024],
                        xbT[ct * 128:(ct + 1) * 128, tch * 1024:(tch + 1) * 1024])
            woTs = cpool.tile([128, CT, C], F32R, tag="woT")       # 8KB
            for ct in range(CT):
                nc.sync.dma_start(woTs[:, ct, :], woT[ct * 128:(ct + 1) * 128, :])
            bias_s = cpool.tile([128, 3, NP], F32, tag="bias")
            nc.sync.dma_start(bias_s[:], bias[:])
            # ind / bo live at partition 64 so matmul operand bases match the
            # rowsum row (PSUM partition 64) they pair with.
            inds = cpool.tile([65, 3, 128], F32R, tag="ind")
            nc.sync.dma_start(inds[64:65, :, :],
                              ind.rearrange("(o a) b -> o a b", o=1))
            bos = cpool.tile([65, C], F32R, tag="bo")
            nc.sync.dma_start(bos[64:65, :], bo[:])
            acat = bpool.tile([128, NP, TQ], F32R, tag="acat")     # 16KB

            # ---- V projection for one pair-group (2 pairs = 4 heads) ----
            # v2p[:, j*4 + pi*2 + h, 0:64] = V rows, col 64 = ones
            def v_proj(pg):
                v2p = bpool.tile([128, KT * 4, 65], F32R, tag="v2p")  # 33KB
                nc.sync.dma_start(v2p[:, :, 64], ones[:])
                wvs = bpool.tile([128, CT, 256], F32R, tag="wvs")
                for ct in range(CT):
                    nc.sync.dma_start(
                        wvs[:, ct, :],
                        wvT[ct * 128:(ct + 1) * 128, pg * 256:(pg + 1) * 256])
                for j in range(KT):
                    pv = psprj.tile([128, 1024], F32, tag=prjtag)
                    for ct in range(CT):
                        nc.tensor.matmul(
                            pv[:, 0:256],
                            xT[:, ct, j * 128:(j + 1) * 128],
                            wvs[:, ct, :],
                            start=(ct == 0), stop=(ct == CT - 1))
                    nc.vector.tensor_copy(
                        v2p[:, j * 4:(j + 1) * 4, 0:64],
                        pv[:, 0:256].rearrange("p (a b) -> p a b", b=64))
                return v2p

            def projs(p):
                # --- K^T projection: [128 d, 4096 t], bias bk per-partition ---
                kT = ktpool.tile([128, T], F32R, tag="kT")
                wks = bpool.tile([128, CT, 128], F32R, tag="wks")
                for ct in range(CT):
                    nc.sync.dma_start(
                        wks[:, ct, :],
                        wkT[ct * 128:(ct + 1) * 128, p * 128:(p + 1) * 128])
                for tp in range(4):
                    pk = psprj.tile([128, 1024], F32, tag=prjtag)
                    for half in range(2):
                        tch = tp * 2 + half
                        for ct in range(CT):
                            nc.tensor.matmul(
                                pk[:, half * 512:(half + 1) * 512],
                                wks[:, ct, :],
                                xT[:, ct, tch * 512:(tch + 1) * 512],
                                start=(ct == 0), stop=(ct == CT - 1))
                    nc.vector.tensor_scalar_add(
                        kT[:, tp * 1024:(tp + 1) * 1024], pk[:],
                        bias_s[:, 1, p:p + 1])
                # --- Q^T projection: [128 d, 1024 q], scale 1/8, bias bq/8 ---
                qT = ktpool.tile([128, TQ], F32R, tag="qT")
                wqs = bpool.tile([128, CT, 128], F32R, tag="wqs")
                for ct in range(CT):
                    nc.sync.dma_start(
                        wqs[:, ct, :],
                        wqT[ct * 128:(ct + 1) * 128, p * 128:(p + 1) * 128])
                pq = psprj.tile([128, 1024], F32, tag=prjtag)
                for half in range(2):
                    for ct in range(CT):
                        nc.tensor.matmul(
                            pq[:, half * 512:(half + 1) * 512],
                            wqs[:, ct, :],
                            xT[:, ct, half * 512:(half + 1) * 512],
                            start=(ct == 0), stop=(ct == CT - 1))
                nc.scalar.activation(qT[:], pq[:], AF.Identity,
                                     bias=bias_s[:, 0, p:p + 1], scale=0.125)
                return kT, qT

            def attn_pair(p, v2p, pi, kT, qT):
                # --- attention: scores^T -> exp -> attn@[V|1] accumulate ---
                stage = bpool.tile([65, 2, TQ], F32R, tag="stage")   # 8KB
                odd = wpool.tile([64, TQ], F32R, tag="sc")
                for h in range(2):
                    av = psav.tile([65, TQ], F32, tag="av" if av_single else f"av{h}")
                    d0 = h * 64
                    # pipeline unit = 3 half-tiles (1.5 k-tiles) to amortize
                    # the ACT per-instruction overhead; accumulation flags
                    # stay keyed on the k-tile index per 512-col PSUM region
                    Hh = 0
                    while Hh < 2 * KT:
                        n = min(2, 2 * KT - Hh)
                        pss = ps.tile([128, 512 * n], F32, tag="big")
                        for pos in range(n):
                            j, half = (Hh + pos) // 2, (Hh + pos) % 2
                            nc.tensor.matmul(
                                pss[:, pos * 512:(pos + 1) * 512],
                                kT[d0:d0 + 64, j * 128:(j + 1) * 128],
                                qT[d0:d0 + 64, half * 512:(half + 1) * 512],
                                start=True, stop=True)
                        at = apool.tile([128, 512 * n], F32R, tag="attn")
                        nc.scalar.activation(at[:], pss[:], AF.Exp)
                        for pos in range(n):
                            j, half = (Hh + pos) // 2, (Hh + pos) % 2
                            nc.tensor.matmul(
                                av[:, half * 512:(half + 1) * 512],
                                v2p[:, j * 4 + pi * 2 + h, :],
                                at[:, pos * 512:(pos + 1) * 512],
                                start=(j == 0), stop=(j == KT - 1))
                        Hh += n
                    # drain this head's accumulator so the next head can
                    # reuse the single PSUM slot
                    nc.vector.tensor_copy(stage[64:65, h, :], av[64:65, :])
                    if h == 0:
                        nc.vector.tensor_copy(acat[0:64, p, :], av[0:64, :])
                    else:
                        nc.vector.tensor_copy(odd[:], av[0:64, :])
                        nc.sync.dma_start(acat[64:128, p, :], odd[:])

                # --- normalize: U / rowsum + bv  into acat[:, p, :] ---
                pb = psprj.tile([128, 1024], F32, tag=prjtag)
                for half in range(2):
                    for h in range(2):
                        nc.tensor.matmul(
                            pb[:, half * 512:(half + 1) * 512],
                            inds[64:65, h, :],
                            stage[64:65, h, half * 512:(half + 1) * 512],
                            start=(h == 0), stop=(h == 1))
                rb = wpool.tile([128, TQ], F32, tag="sc")
                nc.vector.reciprocal(rb[:], pb[:])
                nc.vector.tensor_tensor(
                    out=acat[:, p, :], in0=acat[:, p, :], in1=rb[:], op=MULT)
                nc.vector.tensor_scalar_add(
                    acat[:, p, :], acat[:, p, :], bias_s[:, 2, p:p + 1])

            kq = projs(0)
            v2p = v_proj(0)
            attn_pair(0, v2p, 0, *kq)
            for p in range(1, NP):
                kq = projs(p)
                if p == 2:
                    v2p = v_proj(1)
                attn_pair(p, v2p, p % 2, *kq)

            # ---- output projection: out[t, :] = acat^T.T @ woT + bo ----
            for qt in range(8):
                po = psprj.tile([128, 1024], F32, tag=prjtag)
                for r in range(CT):
                    nc.tensor.matmul(
                        po[:, 0:512],
                        acat[:, r, qt * 128:(qt + 1) * 128],
                        woTs[:, r, :],
                        start=(r == 0), stop=False)
                nc.tensor.matmul(po[:, 0:512], inds[64:65, 2, :],
                                 bos[64:65, :], start=False, stop=True)
                ot = wpool.tile([128, C], F32, tag="sc")
                nc.vector.tensor_copy(ot[:], po[:, 0:512])
                nc.sync.dma_start(out[qt * 128:(qt + 1) * 128, :], ot[:])

    nc.compile()
    return nc


def _prep_inputs(x, Wq, bq, Wk, bk, Wv, bv, Wo, bo):
    wqT = np.ascontiguousarray(Wq.T)
    wkT = np.ascontiguousarray(Wk.T)
    wvT = np.ascontiguousarray(Wv.T)
    woT = np.ascontiguousarray(Wo.T)
    bias = np.stack([
        (bq / 8.0).reshape(NP, 128).T,
        bk.reshape(NP, 128).T,
        bv.reshape(NP, 128).T,
    ], axis=1).astype(np.float32)          # [128, 3, NP]
    bias = np.ascontiguousarray(bias)
    bo1 = np.ascontiguousarray(bo.reshape(1, C))
    ind = np.zeros((3, 128), np.float32)
    ind[0, 0:64] = 1.0
    ind[1, 64:128] = 1.0
    ind[2, :] = 1.0
    in_maps = []
    for i in range(8):
        b, q0 = i // 4, (i % 4) * TQ
        xbT = np.ascontiguousarray(np.roll(x[b].T, -q0, axis=1))
        in_maps.append({
            "xbT": xbT, "wqT": wqT, "wkT": wkT, "wvT": wvT, "woT": woT,
            "bias": bias, "bo": bo1, "ind": ind,
            "ones": np.ones((128, KT * 4), np.float32),
        })
    return in_maps


def kernel(x, Wq, bq, Wk, bk, Wv, bv, Wo, bo):
    x = np.asarray(x, np.float32)
    args = [np.asarray(a, np.float32) for a in
            (Wq, bq, Wk, bk, Wv, bv, Wo, bo)]
    if "nc" not in _cache:
        _cache["nc"] = _build()
    nc = _cache["nc"]
    in_maps = _prep_inputs(x, *args)
    res = run_bass_kernel_spmd(nc, in_maps, list(range(8)))
    outf = np.empty((B, T, C), np.float32)
    for i in range(8):
        b, q0 = i // 4, (i % 4) * TQ
        outf[b, q0:q0 + TQ, :] = res.results[i]["out"]
    return outf



# revision 22
# speedup vs baseline: 1.3185x; 1.3185x over previous
"""Multi-head attention (B=2, T=4096, D=512, H=8) on 8 Trainium2 cores.

Sharding: core i handles batch b=i//4, query rows q0=(i%4)*1024 .. q0+1024,
all 8 heads (full K/V of its batch computed on-core; no collectives).
Host pre-transposes x and weights (bf16) and rolls x along T per core so
each core's query block sits at columns 0:1024.

v2 pipeline (exp-bound design):
- Projections in bf16 (x, W all bf16; f32 PSUM accumulation).
- Scores via fp8e4(e4m3) DoubleRow matmuls: K^T/Q^T drained to fp8 with a
  zeroed second k-tile slot, so one DR matmul contracts the full d_k=64 at
  0.5 cycles/row.  exp applies the 1/sqrt(d_k)=1/8 scale for free.
- exp on the Activation engine in 3-bank (1536-col) PSUM chunks, double
  buffered through a 6-bank ring; output bf16 `at` tiles.
- AV with swapped operands (stationary=at [128k,128q], moving=V|1 [128k,65])
  accumulating [128q, 65] in a single PSUM bank; per-partition rowsum ->
  reciprocal (DVE) -> normalize to bf16 (GpSimd), then DMA-transpose
  [128q,128d] -> acat [128d, q].
- bv is folded into bo on the host (bo' = bo + bv @ Wo.T), bq/bk folded
  into the fp8 drains.
- Output projection in bf16 + f32r bias matmul, f32 result.
"""
import sys
sys.path.insert(0, "/opt/trn_rl_repo")

import numpy as np
import ml_dtypes
import concourse.bacc as bacc
import concourse.mybir as mybir
import concourse.tile as tile
from concourse.bass_utils import run_bass_kernel_spmd

F32 = mybir.dt.float32
F32R = mybir.dt.float32r
BF16 = mybir.dt.bfloat16
F8 = mybir.dt.float8e4
AF = mybir.ActivationFunctionType
ADD = mybir.AluOpType.add
MULT = mybir.AluOpType.mult
DR = mybir.MatmulPerfMode.DoubleRow

B, T, C = 2, 4096, 512
H, DK = 8, 64
TQ = 1024          # queries per core
NP = 4             # head pairs
KT = T // 128      # 32 k-tiles
CT = C // 128      # 4 contraction tiles
NPH = 2 * H        # 16 phases (head, q-half)

_cache = {}


def _build():
    nc = bacc.Bacc("TRN2")
    xbT = nc.declare_dram_parameter("xbT", [C, T], BF16, isOutput=False)
    wqT = nc.declare_dram_parameter("wqT", [C, C], BF16, isOutput=False)
    wkT = nc.declare_dram_parameter("wkT", [C, C], BF16, isOutput=False)
    wvT = nc.declare_dram_parameter("wvT", [C, C], BF16, isOutput=False)
    woT = nc.declare_dram_parameter("woT", [C, C], BF16, isOutput=False)
    # bias[:, 0, p] = bq slice, bias[:, 1, p] = bk slice
    bias = nc.declare_dram_parameter("bias", [128, 2, NP], F32, isOutput=False)
    bof = nc.declare_dram_parameter("bof", [1, C], F32R, isOutput=False)
    ones1 = nc.declare_dram_parameter("ones1", [1, 128], F32R, isOutput=False)
    out = nc.declare_dram_parameter("out", [TQ, C], F32, isOutput=True)

    with tile.TileContext(nc) as tc:
        with (
            tc.tile_pool(name="big", bufs=1) as bpool,
            tc.tile_pool(name="v2", bufs=2) as v2pool,
            tc.tile_pool(name="rc", bufs=4) as rpool,
            tc.tile_pool(name="ot", bufs=2) as opool,
            tc.tile_pool(name="ring", bufs=2, space="PSUM") as ring,
            tc.tile_pool(name="avp", bufs=1, space="PSUM") as avp,
            tc.tile_pool(name="prj", bufs=1, space="PSUM") as prj,
        ):
            # ---- static SBUF tiles ----
            xT = bpool.tile([128, CT, T], BF16, tag="xT")        # 32KB/part
            woTs = bpool.tile([128, CT, C], BF16, tag="woT")     # 4KB
            biasS = bpool.tile([128, 2, NP], F32, tag="bias")
            onesO = bpool.tile([65, 128], F32R, tag="ones")
            boS = bpool.tile([65, C], F32R, tag="bo")
            # fp8 K^T/Q^T, double-buffered by pair parity; [:,1,:] stays 0
            kf8a = bpool.tile([128, 2, T], F8, tag="kf8a")
            kf8b = bpool.tile([128, 2, T], F8, tag="kf8b")
            qf8a = bpool.tile([128, 2, TQ], F8, tag="qf8a")
            qf8b = bpool.tile([128, 2, TQ], F8, tag="qf8b")
            kf8 = [kf8a, kf8b]
            qf8 = [qf8a, qf8b]
            # exp output, double-buffered by phase parity
            at0 = bpool.tile([128, KT, 512], BF16, tag="at0")    # 32KB
            at1 = bpool.tile([128, KT, 512], BF16, tag="at1")    # 32KB
            at = [at0, at1]
            # normalized AV, [q, d-pair]; double-buffered by pair parity
            avn0 = bpool.tile([128, 8, 128], BF16, tag="avn0")
            avn1 = bpool.tile([128, 8, 128], BF16, tag="avn1")
            avn = [avn0, avn1]
            acat = bpool.tile([128, NP, TQ], BF16, tag="acat")   # 8KB
            oacc = bpool.tile([128, 8, C], F32, tag="oacc")      # 16KB
            wkS = bpool.tile([128, CT, C], BF16, tag="wkS")
            wqS = bpool.tile([128, CT, C], BF16, tag="wqS")
            wvS = bpool.tile([128, CT, C], BF16, tag="wvS")

            # ---- prologue DMAs ----
            xv = xbT.rearrange("(ct p) t -> p ct t", p=128)
            wkv = wkT.rearrange("(ct p) c -> p ct c", p=128)
            wqv = wqT.rearrange("(ct p) c -> p ct c", p=128)
            wvv = wvT.rearrange("(ct p) c -> p ct c", p=128)
            nc.sync.dma_start(wkS[:], wkv[:])
            nc.sync.dma_start(xT[:, :, 0:1024], xv[:, :, 0:1024])
            nc.sync.dma_start(wqS[:], wqv[:])
            nc.sync.dma_start(biasS[:], bias[:])
            nc.sync.dma_start(wvS[:], wvv[:])
            for tch in range(1, 4):
                nc.sync.dma_start(
                    xT[:, :, tch * 1024:(tch + 1) * 1024],
                    xv[:, :, tch * 1024:(tch + 1) * 1024])
            nc.sync.dma_start(onesO[64:65, :],
                              ones1.rearrange("(o a) b -> o a b", o=1))
            nc.sync.dma_start(boS[64:65, :], bof.rearrange("(o a) b -> o a b", o=1))
            nc.sync.dma_start(woTs[:], woT.rearrange("(ct p) c -> p ct c", p=128))
            for i in range(2):
                nc.gpsimd.memset(kf8[i][:, 1, :], 0.0)
                nc.gpsimd.memset(qf8[i][:, 1, :], 0.0)

            # ---- helper emitters (each returns a list of zero-arg thunks) --

            def k_piece(p, piece):
                # K^T cols piece*512:(piece+1)*512 -> kf8[p%2][:, 0, ...]
                pp = prj.tile([128, 512], F32, tag="prj")
                for ct in range(CT):
                    nc.tensor.matmul(
                        pp[:], wkS[:, ct, p * 128:(p + 1) * 128],
                        xT[:, ct, piece * 512:(piece + 1) * 512],
                        start=(ct == 0), stop=(ct == CT - 1))
                nc.vector.tensor_scalar_add(
                    kf8[p % 2][:, 0, piece * 512:(piece + 1) * 512], pp[:],
                    biasS[:, 1, p:p + 1])

            def q_piece(p, piece):
                pp = prj.tile([128, 512], F32, tag="prj")
                for ct in range(CT):
                    nc.tensor.matmul(
                        pp[:], wqS[:, ct, p * 128:(p + 1) * 128],
                        xT[:, ct, piece * 512:(piece + 1) * 512],
                        start=(ct == 0), stop=(ct == CT - 1))
                nc.vector.tensor_scalar_add(
                    qf8[p % 2][:, 0, piece * 512:(piece + 1) * 512], pp[:],
                    biasS[:, 0, p:p + 1])

            def v_piece(v2p, pg, j):
                # V rows for k-tiles j, j+1: two exact-cover PSUM groups in
                # two different banks of one prj allocation is not possible
                # (prj is 1 bank), so run j and j+1 as separate allocations.
                for jj in range(2):
                    pv = prj.tile([128, 512], F32, tag="prj")
                    for ct in range(CT):
                        nc.tensor.matmul(
                            pv[:, 0:256],
                            xT[:, ct, (j + jj) * 128:(j + jj + 1) * 128],
                            wvS[:, ct, pg * 256:(pg + 1) * 256],
                            start=(ct == 0), stop=(ct == CT - 1))
                    nc.vector.tensor_copy(
                        v2p[:, j + jj, :, 0:64],
                        pv[:, 0:256].rearrange("p (h b) -> p h b", b=64))

            def new_v2p():
                v2p = v2pool.tile([128, KT, 4, 65], BF16, tag="v2p")
                nc.gpsimd.memset(v2p[:, :, :, 64], 1.0)
                return v2p

            def av_group(ph, qb, v2p, av_ap=None):
                # AV for phase ph (= head h, q-half), query block qb (0..3)
                h, half = ph // 2, ph % 2
                hb = ph % 2  # at buffer parity
                d0 = (h % 2) * 64
                pb = (h // 2) % 2
                qbg = half * 4 + qb
                if av_ap is None:
                    av_t = avp.tile([128, 512], F32, tag="av")
                else:
                    av_t = av_ap
                for kt in range(KT):
                    nc.tensor.matmul(
                        av_t[:, 0:65],
                        at[hb][:, kt, qb * 128:(qb + 1) * 128],
                        v2p[:, kt, h % 4, :],
                        start=(kt == 0), stop=(kt == KT - 1))
                avr = rpool.tile([128, 65], F32, tag="avr")
                nc.vector.tensor_copy(avr[:], av_t[:, 0:65])
                rec = rpool.tile([128, 1], F32, tag="rec")
                nc.vector.reciprocal_approx_fast(rec[:], avr[:, 64:65])
                nc.vector.tensor_scalar(
                    avn[pb][:, qbg, d0:d0 + 64], avr[:, 0:64],
                    rec[:], None, MULT)

            def o_piece(qt, po=None):
                if po is None:
                    po = prj.tile([128, 512], F32, tag="prj")
                nc.tensor.matmul(
                    po[:], acat[:, 3, qt * 128:(qt + 1) * 128],
                    woTs[:, 3, :], start=True, stop=True)
                ot = opool.tile([128, 512], F32, tag="ot")
                nc.vector.tensor_tensor(out=ot[:], in0=po[:],
                                        in1=oacc[:, qt, :], op=ADD)
                nc.sync.dma_start(out[qt * 128:(qt + 1) * 128, :], ot[:])

            # ---- prologue compute: pair-0 K/Q on ring slots (batched drains) --

            def ring_kq(groups):
                """groups: list of ('k'|'q', p, piece). One ring tile, one
                matmul group per bank, batched drains per contiguous run."""
                rt = ring.tile([128, 1536], F32, tag="ring")
                for g, (kind, p, piece) in enumerate(groups):
                    w = wkS if kind == "k" else wqS
                    for ct in range(CT):
                        nc.tensor.matmul(
                            rt[:, g * 512:(g + 1) * 512],
                            w[:, ct, p * 128:(p + 1) * 128],
                            xT[:, ct, piece * 512:(piece + 1) * 512],
                            start=(ct == 0), stop=(ct == CT - 1))
                # batched drains over contiguous same-kind runs
                g = 0
                while g < len(groups):
                    kind, p, piece = groups[g]
                    g2 = g
                    while (g2 + 1 < len(groups)
                           and groups[g2 + 1][0] == kind
                           and groups[g2 + 1][2] == groups[g2][2] + 1):
                        g2 += 1
                    dst = kf8[p % 2] if kind == "k" else qf8[p % 2]
                    bcol = 1 if kind == "k" else 0
                    nc.vector.tensor_scalar_add(
                        dst[:, 0, piece * 512:piece * 512 + (g2 - g + 1) * 512],
                        rt[:, g * 512:(g2 + 1) * 512],
                        biasS[:, bcol, p:p + 1])
                    g = g2 + 1

            ring_kq([("k", 0, 0), ("q", 0, 0)])
            ring_kq([("q", 0, 1), ("k", 0, 1), ("k", 0, 2)])
            ring_kq([("k", 0, 3), ("k", 0, 4), ("k", 0, 5)])
            v2p_cur = new_v2p()

            # ---- main pipeline over 16 phases ----
            state = {"v2p": v2p_cur, "v2p_next": None, "pending": []}

            def phase_background(ph):
                """Thunks to interleave into phase ph's chunk stream."""
                thunks = []
                h, half = ph // 2, ph % 2
                p = h // 2
                # deferred transposes/O-pieces from the previous phase first
                pend, state["pending"] = state["pending"], []
                thunks.extend(pend)
                # leftover pg0 V-pieces MUST precede phase-0's AV groups
                if ph == 1:
                    for j in range(22, KT, 2):
                        thunks.append(lambda j=j: v_piece(state["v2p"], 0, j))
                # AV of previous phase (+ deferred transpose & O-proj)
                if ph >= 1:
                    prev_h = (ph - 1) // 2
                    v2p_prev = (state["v2p_prev4"] if prev_h // 4 != h // 4
                                else state["v2p"])
                    for qb in range(4):
                        thunks.append(lambda ph=ph, qb=qb, v=v2p_prev:
                                      av_tr_o(ph - 1, qb, v))
                # projection prep for pair p+1
                slot = ph % 4
                if p + 1 < NP:
                    if slot == 1:
                        for piece in range(4):
                            thunks.append(lambda p=p, piece=piece:
                                          k_piece(p + 1, piece))
                    elif slot == 2:
                        for piece in range(4, 8):
                            thunks.append(lambda p=p, piece=piece:
                                          k_piece(p + 1, piece))
                    elif slot == 3:
                        for piece in range(2):
                            thunks.append(lambda p=p, piece=piece:
                                          q_piece(p + 1, piece))
                # pair-0 K piece 7 early in phase 0
                if ph == 0:
                    thunks.append(lambda: k_piece(0, 6))
                    thunks.append(lambda: k_piece(0, 7))
                    for j in range(0, 22, 2):
                        thunks.append(lambda j=j: v_piece(state["v2p"], 0, j))
                if 4 <= ph <= 7:
                    if ph == 4:
                        def mkv():
                            state["v2p_next"] = new_v2p()
                        thunks.append(mkv)
                    for j in range((ph - 4) * 8, (ph - 4) * 8 + 8, 2):
                        thunks.append(lambda j=j: v_piece(state["v2p_next"],
                                                          1, j))
                if ph == 13:
                    for qt in range(4):
                        thunks.append(lambda qt=qt: o_partial(qt))
                if ph == 14:
                    for qt in range(4, 8):
                        thunks.append(lambda qt=qt: o_partial(qt))
                return thunks

            def o_partial(qt):
                po = prj.tile([128, 512], F32, tag="prj")
                for r in range(3):
                    nc.tensor.matmul(
                        po[:], acat[:, r, qt * 128:(qt + 1) * 128],
                        woTs[:, r, :], start=(r == 0), stop=False)
                nc.tensor.matmul(po[:], onesO[64:65, :], boS[64:65, :],
                                 start=False, stop=True)
                nc.vector.tensor_copy(oacc[:, qt, :], po[:])

            def tr_o(p, qbg, po=None):
                nc.sync.dma_start_transpose(
                    acat[:, p, qbg * 128:(qbg + 1) * 128],
                    avn[p % 2][:, qbg, :])
                if p == NP - 1:
                    o_piece(qbg, po)

            def av_tr_o(ph, qb, v2p, av_ap=None, po=None, defer=True):
                """AV group + (for odd heads) transpose + (pair 3) O-proj."""
                av_group(ph, qb, v2p, av_ap)
                h, half = ph // 2, ph % 2
                if h % 2 == 1:
                    p = h // 2
                    qbg = half * 4 + qb
                    if defer:
                        state["pending"].append(
                            lambda p=p, qbg=qbg, po=po: tr_o(p, qbg, po))
                    else:
                        tr_o(p, qbg, po)

            for ph in range(NPH):
                h, half = ph // 2, ph % 2
                if ph == 8:
                    state["v2p_prev4"] = state["v2p"]
                    state["v2p"] = state["v2p_next"]
                state.setdefault("v2p_prev4", state["v2p"])
                d0 = (h % 2) * 64
                kcur, qcur = kf8[h // 2 % 2], qf8[h // 2 % 2]
                bg = phase_background(ph)
                bgi = 0
                # 11 chunks: 10x3 kt + 1x2 kt
                for c in range(11):
                    n = 3 if c < 10 else 2
                    ring_t = ring.tile([128, 1536], F32, tag="ring")
                    for jj in range(n):
                        kt = 3 * c + jj
                        nc.tensor.matmul(
                            ring_t[:, jj * 512:(jj + 1) * 512],
                            kcur[d0:d0 + 64, :, kt * 128:(kt + 1) * 128],
                            qcur[d0:d0 + 64, :, half * 512:(half + 1) * 512],
                            start=True, stop=True, perf_mode=DR,
                            tile_position=(d0, 0))
                    nc.scalar.activation(
                        at[ph % 2][:, 3 * c:3 * c + n, :],
                        ring_t[:, 0:512 * n].rearrange("p (a b) -> p a b", b=512),
                        AF.Exp, scale=0.125)
                    # interleave background work: spread across chunks
                    n_bg = (len(bg) * (c + 1)) // 11 - (len(bg) * c) // 11
                    for _ in range(n_bg):
                        bg[bgi]()
                        bgi += 1
                assert bgi == len(bg)

            # ---- epilogue: AV of phase 15 + pair-3 transposes + O-proj ----
            # ring banks are free: give every AV group and O-piece its own
            # bank and software-pipeline so no PE wait blocks later work.
            for th in state["pending"]:
                th()
            h15 = (NPH - 1) // 2
            v2p15 = state["v2p"]

            def av15_pair(qbs, tiles):
                for kt in range(KT):
                    for qb, t in zip(qbs, tiles):
                        nc.tensor.matmul(
                            t[:, 0:65],
                            at[(NPH - 1) % 2][:, kt, qb * 128:(qb + 1) * 128],
                            v2p15[:, kt, h15 % 4, :],
                            start=(kt == 0), stop=(kt == KT - 1))

            def norm_tr(qb, t):
                qbg = 4 + qb
                avr = rpool.tile([128, 65], F32, tag="avr")
                nc.vector.tensor_copy(avr[:], t[:, 0:65])
                rec = rpool.tile([128, 1], F32, tag="rec")
                nc.vector.reciprocal_approx_fast(rec[:], avr[:, 64:65])
                nc.vector.tensor_scalar(
                    avn[1][:, qbg, 64:128], avr[:, 0:64], rec[:], None, MULT)
                nc.sync.dma_start_transpose(
                    acat[:, 3, qbg * 128:(qbg + 1) * 128],
                    avn[1][:, qbg, :])

            avA = avp.tile([128, 512], F32, tag="av")
            avB = prj.tile([128, 512], F32, tag="prj")
            av15_pair([0, 1], [avA, avB])
            norm_tr(0, avA)
            norm_tr(1, avB)
            avA2 = avp.tile([128, 512], F32, tag="av")
            avB2 = prj.tile([128, 512], F32, tag="prj")
            av15_pair([2, 3], [avA2, avB2])
            norm_tr(2, avA2)
            norm_tr(3, avB2)
            epo = ring.tile([128, 1536], F32, tag="ring")
            epo2 = ring.tile([128, 1536], F32, tag="ring")
            for i, qt in enumerate([4, 5, 6, 7]):
                po = epo[:, i * 512:(i + 1) * 512] if i < 3 else epo2[:, 0:512]
                o_piece(qt, po=po)

    nc.compile()
    return nc


def _prep_inputs(x, Wq, bq, Wk, bk, Wv, bv, Wo, bo):
    bf = ml_dtypes.bfloat16
    wqT = np.ascontiguousarray(Wq.T).astype(bf)
    wkT = np.ascontiguousarray(Wk.T).astype(bf)
    wvT = np.ascontiguousarray(Wv.T).astype(bf)
    woT = np.ascontiguousarray(Wo.T).astype(bf)
    bias = np.stack([
        bq.reshape(NP, 128).T,
        bk.reshape(NP, 128).T,
    ], axis=1).astype(np.float32)          # [128, 2, NP]
    bias = np.ascontiguousarray(bias)
    bof = np.ascontiguousarray(
        (bo.astype(np.float64) + bv.astype(np.float64) @ Wo.astype(np.float64).T)
        .reshape(1, C)).astype(np.float32)
    ones1 = np.ones((1, 128), np.float32)
    in_maps = []
    for i in range(8):
        b, q0 = i // 4, (i % 4) * TQ
        xbT = np.ascontiguousarray(np.roll(x[b].T, -q0, axis=1)).astype(bf)
        in_maps.append({
            "xbT": xbT, "wqT": wqT, "wkT": wkT, "wvT": wvT, "woT": woT,
            "bias": bias, "bof": bof, "ones1": ones1,
        })
    return in_maps


def kernel(x, Wq, bq, Wk, bk, Wv, bv, Wo, bo):
    x = np.asarray(x, np.float32)
    args = [np.asarray(a, np.float32) for a in
            (Wq, bq, Wk, bk, Wv, bv, Wo, bo)]
    if "nc" not in _cache:
        _cache["nc"] = _build()
    nc = _cache["nc"]
    in_maps = _prep_inputs(x, *args)
    res = run_bass_kernel_spmd(nc, in_maps, list(range(8)))
    outf = np.empty((B, T, C), np.float32)
    for i in range(8):
        b, q0 = i // 4, (i % 4) * TQ
        outf[b, q0:q0 + TQ, :] = res.results[i]["out"]
    return outf


# revision 30
# speedup vs baseline: 1.3649x; 1.0351x over previous
"""Multi-head attention (B=2, T=4096, D=512, H=8) on 8 Trainium2 cores.

Sharding: core i handles batch b=i//4, query rows q0=(i%4)*1024 .. q0+1024,
all 8 heads (full K/V of its batch computed on-core; no collectives).
Host pre-transposes x and weights (bf16) and rolls x along T per core so
each core's query block sits at columns 0:1024.

v2 pipeline (exp-bound design):
- Projections in bf16 (x, W all bf16; f32 PSUM accumulation).
- Scores via fp8e4(e4m3) DoubleRow matmuls: K^T/Q^T drained to fp8 with a
  zeroed second k-tile slot, so one DR matmul contracts the full d_k=64 at
  0.5 cycles/row.  exp applies the 1/sqrt(d_k)=1/8 scale for free.
- exp on the Activation engine in 3-bank (1536-col) PSUM chunks, double
  buffered through a 6-bank ring; output bf16 `at` tiles.
- AV with swapped operands (stationary=at [128k,128q], moving=V|1 [128k,65])
  accumulating [128q, 65] in a single PSUM bank; per-partition rowsum ->
  reciprocal (DVE) -> normalize to bf16 (GpSimd), then DMA-transpose
  [128q,128d] -> acat [128d, q].
- bv is folded into bo on the host (bo' = bo + bv @ Wo.T), bq/bk folded
  into the fp8 drains.
- Output projection in bf16 + f32r bias matmul, f32 result.
"""
import sys
sys.path.insert(0, "/opt/trn_rl_repo")

import numpy as np
import ml_dtypes
import concourse.bacc as bacc
import concourse.mybir as mybir
import concourse.tile as tile
from concourse.bass_utils import run_bass_kernel_spmd

F32 = mybir.dt.float32
F32R = mybir.dt.float32r
BF16 = mybir.dt.bfloat16
F8 = mybir.dt.float8e4
AF = mybir.ActivationFunctionType
ADD = mybir.AluOpType.add
MULT = mybir.AluOpType.mult
DR = mybir.MatmulPerfMode.DoubleRow

B, T, C = 2, 4096, 512
H, DK = 8, 64
TQ = 1024          # queries per core
NP = 4             # head pairs
KT = T // 128      # 32 k-tiles
CT = C // 128      # 4 contraction tiles
NPH = 2 * H        # 16 phases (head, q-half)

_cache = {}


def _build():
    nc = bacc.Bacc("TRN2")
    xbT = nc.declare_dram_parameter("xbT", [C, T], BF16, isOutput=False)
    wqT = nc.declare_dram_parameter("wqT", [C, C], BF16, isOutput=False)
    wkT = nc.declare_dram_parameter("wkT", [C, C], BF16, isOutput=False)
    wvT = nc.declare_dram_parameter("wvT", [C, C], BF16, isOutput=False)
    woT = nc.declare_dram_parameter("woT", [C, C], BF16, isOutput=False)
    # bias[:, 0, p] = bq slice, bias[:, 1, p] = bk slice
    bias = nc.declare_dram_parameter("bias", [128, 2, NP], F32, isOutput=False)
    bof = nc.declare_dram_parameter("bof", [1, C], F32R, isOutput=False)
    ones1 = nc.declare_dram_parameter("ones1", [1, 128], F32R, isOutput=False)
    out = nc.declare_dram_parameter("out", [TQ, C], F32, isOutput=True)

    with tile.TileContext(nc) as tc:
        with (
            tc.tile_pool(name="big", bufs=1) as bpool,
            tc.tile_pool(name="v2", bufs=2) as v2pool,
            tc.tile_pool(name="rc", bufs=4) as rpool,
            tc.tile_pool(name="ot", bufs=2) as opool,
            tc.tile_pool(name="ring", bufs=2, space="PSUM") as ring,
            tc.tile_pool(name="avp", bufs=1, space="PSUM") as avp,
            tc.tile_pool(name="prj", bufs=1, space="PSUM") as prj,
        ):
            # ---- static SBUF tiles ----
            xT = bpool.tile([128, CT, T], BF16, tag="xT")        # 32KB/part
            woTs = bpool.tile([128, CT, C], BF16, tag="woT")     # 4KB
            biasS = bpool.tile([128, 2, NP], F32, tag="bias")
            onesO = bpool.tile([65, 128], F32R, tag="ones")
            boS = bpool.tile([65, C], F32R, tag="bo")
            # fp8 K^T/Q^T, double-buffered by pair parity; [:,1,:] stays 0
            kf8a = bpool.tile([128, 2, T], F8, tag="kf8a")
            kf8b = bpool.tile([128, 2, T], F8, tag="kf8b")
            qf8a = bpool.tile([128, 2, TQ], F8, tag="qf8a")
            qf8b = bpool.tile([128, 2, TQ], F8, tag="qf8b")
            kf8 = [kf8a, kf8b]
            qf8 = [qf8a, qf8b]
            # exp output, double-buffered by phase parity
            at0 = bpool.tile([128, KT, 512], BF16, tag="at0")    # 32KB
            at1 = bpool.tile([128, KT, 512], BF16, tag="at1")    # 32KB
            at = [at0, at1]
            # normalized AV, [q, d-pair]; double-buffered by pair parity
            avn0 = bpool.tile([128, 8, 128], BF16, tag="avn0")
            avn1 = bpool.tile([128, 8, 128], BF16, tag="avn1")
            avn = [avn0, avn1]
            acat = bpool.tile([128, NP, TQ], BF16, tag="acat")   # 8KB
            oacc = bpool.tile([128, 8, C], F32, tag="oacc")      # 16KB
            wkS = bpool.tile([128, CT, C], BF16, tag="wkS")
            wqS = bpool.tile([128, CT, C], BF16, tag="wqS")
            wvS = bpool.tile([128, CT, C], BF16, tag="wvS")

            # ---- prologue DMAs ----
            xv = xbT.rearrange("(ct p) t -> p ct t", p=128)
            wkv = wkT.rearrange("(ct p) c -> p ct c", p=128)
            wqv = wqT.rearrange("(ct p) c -> p ct c", p=128)
            wvv = wvT.rearrange("(ct p) c -> p ct c", p=128)
            nc.sync.dma_start(wkS[:], wkv[:])
            nc.sync.dma_start(xT[:, :, 0:1024], xv[:, :, 0:1024])
            nc.sync.dma_start(wqS[:], wqv[:])
            nc.sync.dma_start(biasS[:], bias[:])
            nc.sync.dma_start(wvS[:], wvv[:])
            for tch in range(1, 4):
                nc.sync.dma_start(
                    xT[:, :, tch * 1024:(tch + 1) * 1024],
                    xv[:, :, tch * 1024:(tch + 1) * 1024])
            nc.sync.dma_start(onesO[64:65, :],
                              ones1.rearrange("(o a) b -> o a b", o=1))
            nc.sync.dma_start(boS[64:65, :], bof.rearrange("(o a) b -> o a b", o=1))
            nc.sync.dma_start(woTs[:], woT.rearrange("(ct p) c -> p ct c", p=128))
            dz = bpool.tile([64, 2, 512], F8, tag="dz")
            nc.vector.memset(dz[:], 0.0)
            nc.vector.memset(kf8[0][:, 1, 0:1536], 0.0)
            nc.vector.memset(qf8[0][:, 1, :], 0.0)
            nc.gpsimd.memset(kf8[0][:, 1, 1536:T], 0.0)
            nc.gpsimd.memset(kf8[1][:, 1, :], 0.0)
            nc.gpsimd.memset(qf8[1][:, 1, :], 0.0)

            # ---- helper emitters (each returns a list of zero-arg thunks) --

            def k_piece(p, piece):
                # K^T cols piece*512:(piece+1)*512 -> kf8[p%2][:, 0, ...]
                pp = prj.tile([128, 512], F32, tag="prj")
                for ct in range(CT):
                    nc.tensor.matmul(
                        pp[:], wkS[:, ct, p * 128:(p + 1) * 128],
                        xT[:, ct, piece * 512:(piece + 1) * 512],
                        start=(ct == 0), stop=(ct == CT - 1))
                nc.vector.tensor_scalar_add(
                    kf8[p % 2][:, 0, piece * 512:(piece + 1) * 512], pp[:],
                    biasS[:, 1, p:p + 1])

            def q_piece(p, piece):
                pp = prj.tile([128, 512], F32, tag="prj")
                for ct in range(CT):
                    nc.tensor.matmul(
                        pp[:], wqS[:, ct, p * 128:(p + 1) * 128],
                        xT[:, ct, piece * 512:(piece + 1) * 512],
                        start=(ct == 0), stop=(ct == CT - 1))
                nc.vector.tensor_scalar_add(
                    qf8[p % 2][:, 0, piece * 512:(piece + 1) * 512], pp[:],
                    biasS[:, 0, p:p + 1])

            def v_piece(v2p, pg, j, pool=None):
                # V rows for k-tiles j, j+1 as separate exact-cover groups.
                for jj in range(2):
                    pv = (pool or prj).tile([128, 512], F32,
                                            tag="av" if pool is avp else "prj")
                    for ct in range(CT):
                        nc.tensor.matmul(
                            pv[:, 0:256],
                            xT[:, ct, (j + jj) * 128:(j + jj + 1) * 128],
                            wvS[:, ct, pg * 256:(pg + 1) * 256],
                            start=(ct == 0), stop=(ct == CT - 1))
                    nc.vector.tensor_copy(
                        v2p[:, j + jj, :, 0:64],
                        pv[:, 0:256].rearrange("p (h b) -> p h b", b=64))

            def new_v2p():
                v2p = v2pool.tile([128, KT, 4, 65], BF16, tag="v2p")
                nc.gpsimd.memset(v2p[:, :, :, 64], 1.0)
                return v2p

            def av_group(ph, qb, v2p, av_ap=None):
                # AV for phase ph (= head h, q-half), query block qb (0..3)
                h, half = ph // 2, ph % 2
                hb = ph % 2  # at buffer parity
                d0 = (h % 2) * 64
                pb = (h // 2) % 2
                qbg = half * 4 + qb
                if av_ap is None:
                    av_t = avp.tile([128, 512], F32, tag="av")
                else:
                    av_t = av_ap
                for kt in range(KT):
                    nc.tensor.matmul(
                        av_t[:, 0:65],
                        at[hb][:, kt, qb * 128:(qb + 1) * 128],
                        v2p[:, kt, h % 4, :],
                        start=(kt == 0), stop=(kt == KT - 1))
                avr = rpool.tile([128, 65], F32, tag="avr")
                nc.vector.tensor_copy(avr[:], av_t[:, 0:65])
                rec = rpool.tile([128, 1], F32, tag="rec")
                nc.vector.reciprocal_approx_fast(rec[:], avr[:, 64:65])
                nc.vector.tensor_scalar(
                    avn[pb][:, qbg, d0:d0 + 64], avr[:, 0:64],
                    rec[:], None, MULT)

            def o_piece(qt, po=None):
                if po is None:
                    po = prj.tile([128, 512], F32, tag="prj")
                nc.tensor.matmul(
                    po[:], acat[:, 3, qt * 128:(qt + 1) * 128],
                    woTs[:, 3, :], start=True, stop=True)
                ot = opool.tile([128, 512], F32, tag="ot")
                nc.vector.tensor_tensor(out=ot[:], in0=po[:],
                                        in1=oacc[:, qt, :], op=ADD)
                nc.sync.dma_start(out[qt * 128:(qt + 1) * 128, :], ot[:])

            # ---- prologue compute: pair-0 K/Q on ring slots (batched drains) --

            def ring_kq(groups):
                """groups: list of ('k'|'q', p, piece). One ring tile, one
                matmul group per bank, batched drains per contiguous run."""
                rt = ring.tile([128, 1536], F32, tag="ring")
                for g, (kind, p, piece) in enumerate(groups):
                    w = wkS if kind == "k" else wqS
                    for ct in range(CT):
                        nc.tensor.matmul(
                            rt[:, g * 512:(g + 1) * 512],
                            w[:, ct, p * 128:(p + 1) * 128],
                            xT[:, ct, piece * 512:(piece + 1) * 512],
                            start=(ct == 0), stop=(ct == CT - 1))
                # batched drains over contiguous same-kind runs
                g = 0
                while g < len(groups):
                    kind, p, piece = groups[g]
                    g2 = g
                    while (g2 + 1 < len(groups)
                           and groups[g2 + 1][0] == kind
                           and groups[g2 + 1][2] == groups[g2][2] + 1):
                        g2 += 1
                    dst = kf8[p % 2] if kind == "k" else qf8[p % 2]
                    bcol = 1 if kind == "k" else 0
                    nc.vector.tensor_scalar_add(
                        dst[:, 0, piece * 512:piece * 512 + (g2 - g + 1) * 512],
                        rt[:, g * 512:(g2 + 1) * 512],
                        biasS[:, bcol, p:p + 1])
                    g = g2 + 1

            # PE p-state warm-up on zeros while x loads
            wup = avp.tile([128, 512], F32, tag="av")
            for i in range(14):
                nc.tensor.matmul(wup[:], dz[:, :, 0:128], dz[:],
                                 start=True, stop=True, perf_mode=DR,
                                 tile_position=(0, 0))
            ring_kq([("k", 0, 0)])
            q_piece(0, 0)
            ring_kq([("q", 0, 1), ("k", 0, 1), ("k", 0, 2)])
            ring_kq([("k", 0, 3), ("k", 0, 4), ("k", 0, 5)])
            v2p_cur = new_v2p()

            # ---- main pipeline over 16 phases ----
            state = {"v2p": v2p_cur, "v2p_next": None, "pending": [],
                     "o_pending": []}

            def phase_background(ph):
                """Thunks to interleave into phase ph's chunk stream."""
                thunks = []
                h, half = ph // 2, ph % 2
                p = h // 2
                # deferred transposes/O-pieces from the previous phase first
                pend, state["pending"] = state["pending"], []
                thunks.extend(pend)
                # leftover pg0 V-pieces MUST precede phase-0's AV groups
                if ph == 1:
                    for j in range(28, KT, 2):
                        thunks.append(lambda j=j: v_piece(state["v2p"], 0, j))
                # AV of previous phase (+ deferred transpose & O-proj)
                if ph >= 1:
                    prev_h = (ph - 1) // 2
                    v2p_prev = (state["v2p_prev4"] if prev_h // 4 != h // 4
                                else state["v2p"])
                    for qb in range(4):
                        thunks.append(lambda ph=ph, qb=qb, v=v2p_prev:
                                      av_tr_o(ph - 1, qb, v))
                # projection prep for pair p+1
                slot = ph % 4
                if p + 1 < NP:
                    if slot == 2:
                        for piece in range(5):
                            thunks.append(lambda p=p, piece=piece:
                                          k_piece(p + 1, piece))
                    elif slot == 3:
                        for piece in range(5, 8):
                            thunks.append(lambda p=p, piece=piece:
                                          k_piece(p + 1, piece))
                        for piece in range(2):
                            thunks.append(lambda p=p, piece=piece:
                                          q_piece(p + 1, piece))
                # pair-0 K piece 7 early in phase 0
                if ph == 0:
                    thunks.append(lambda: k_piece(0, 6))
                    thunks.append(lambda: k_piece(0, 7))
                    for i, j in enumerate(range(0, 28, 2)):
                        thunks.append(lambda j=j, i=i: v_piece(
                            state["v2p"], 0, j,
                            pool=avp if i % 2 else prj))
                if 4 <= ph <= 7:
                    if ph == 4:
                        def mkv():
                            state["v2p_next"] = new_v2p()
                        thunks.append(mkv)
                    for j in range((ph - 4) * 8, (ph - 4) * 8 + 8, 2):
                        thunks.append(lambda j=j: v_piece(state["v2p_next"],
                                                          1, j))
                if ph == 13:
                    for qt in range(4):
                        thunks.append(lambda qt=qt: o_partial(qt))
                if ph == 14:
                    for qt in range(4, 8):
                        thunks.append(lambda qt=qt: o_partial(qt))
                return thunks

            def o_partial(qt):
                po = prj.tile([128, 512], F32, tag="prj")
                for r in range(3):
                    nc.tensor.matmul(
                        po[:], acat[:, r, qt * 128:(qt + 1) * 128],
                        woTs[:, r, :], start=(r == 0), stop=False)
                nc.tensor.matmul(po[:], onesO[64:65, :], boS[64:65, :],
                                 start=False, stop=True)
                nc.vector.tensor_copy(oacc[:, qt, :], po[:])

            def tr_o(p, qbg, po=None):
                nc.sync.dma_start_transpose(
                    acat[:, p, qbg * 128:(qbg + 1) * 128],
                    avn[p % 2][:, qbg, :])
                if p == NP - 1:
                    o_piece(qbg, po)

            def av_tr_o(ph, qb, v2p, av_ap=None, po=None, defer=True):
                """AV group + (for odd heads) transpose + (pair 3) O-proj."""
                av_group(ph, qb, v2p, av_ap)
                h, half = ph // 2, ph % 2
                if h % 2 == 1:
                    p = h // 2
                    qbg = half * 4 + qb
                    if p == NP - 1:
                        # last pair: transpose deferred, O-piece to epilogue
                        state["pending"].append(
                            lambda qbg=qbg: nc.sync.dma_start_transpose(
                                acat[:, 3, qbg * 128:(qbg + 1) * 128],
                                avn[1][:, qbg, :]))
                        state["o_pending"].append(qbg)
                    elif defer:
                        state["pending"].append(
                            lambda p=p, qbg=qbg, po=po: tr_o(p, qbg, po))
                    else:
                        tr_o(p, qbg, po)

            for ph in range(NPH):
                h, half = ph // 2, ph % 2
                if ph == 8:
                    state["v2p_prev4"] = state["v2p"]
                    state["v2p"] = state["v2p_next"]
                state.setdefault("v2p_prev4", state["v2p"])
                d0 = (h % 2) * 64
                kcur, qcur = kf8[h // 2 % 2], qf8[h // 2 % 2]
                bg = phase_background(ph)
                bgi = 0
                # 11 chunks: 10x3 kt + 1x2 kt
                for c in range(11):
                    n = 3 if c < 10 else 2
                    ring_t = ring.tile([128, 1536], F32, tag="ring")
                    for jj in range(n):
                        kt = 3 * c + jj
                        nc.tensor.matmul(
                            ring_t[:, jj * 512:(jj + 1) * 512],
                            kcur[d0:d0 + 64, :, kt * 128:(kt + 1) * 128],
                            qcur[d0:d0 + 64, :, half * 512:(half + 1) * 512],
                            start=True, stop=True, perf_mode=DR,
                            tile_position=(d0, 0))
                    nc.scalar.activation(
                        at[ph % 2][:, 3 * c:3 * c + n, :],
                        ring_t[:, 0:512 * n].rearrange("p (a b) -> p a b", b=512),
                        AF.Exp, scale=0.125)
                    # interleave background work: spread across chunks
                    n_bg = (len(bg) * (c + 1)) // 11 - (len(bg) * c) // 11
                    for _ in range(n_bg):
                        bg[bgi]()
                        bgi += 1
                assert bgi == len(bg)
                if ph == NPH - 1:
                    # flush pair-3 transposes first, then their O-pieces
                    pend, state["pending"] = state["pending"], []
                    for th in pend:
                        th()
                    for qt in state["o_pending"][:4]:
                        o_piece(qt)
                    state["o_pending"] = state["o_pending"][4:]

            # ---- epilogue: AV of phase 15 + pair-3 transposes + O-proj ----
            # ring banks are free: give every AV group and O-piece its own
            # bank and software-pipeline so no PE wait blocks later work.
            for th in state["pending"]:
                th()
            h15 = (NPH - 1) // 2
            v2p15 = state["v2p"]
            avA = avp.tile([128, 512], F32, tag="av")
            epA = ring.tile([128, 1536], F32, tag="ring")
            av_aps = [avA, epA[:, 0:512], epA[:, 512:1024], epA[:, 1024:1536]]
            for kt in range(KT):
                for qb in range(4):
                    nc.tensor.matmul(
                        av_aps[qb][:, 0:65],
                        at[(NPH - 1) % 2][:, kt, qb * 128:(qb + 1) * 128],
                        v2p15[:, kt, h15 % 4, :],
                        start=(kt == 0), stop=(kt == KT - 1))

            def norm_tr(qb, t):
                qbg = 4 + qb
                avr = rpool.tile([128, 65], F32, tag="avr")
                nc.vector.tensor_copy(avr[:], t[:, 0:65])
                rec = rpool.tile([128, 1], F32, tag="rec")
                nc.vector.reciprocal_approx_fast(rec[:], avr[:, 64:65])
                nc.vector.tensor_scalar(
                    avn[1][:, qbg, 64:128], avr[:, 0:64], rec[:], None, MULT)
                nc.sync.dma_start_transpose(
                    acat[:, 3, qbg * 128:(qbg + 1) * 128],
                    avn[1][:, qbg, :])

            for qb in range(4):
                norm_tr(qb, av_aps[qb])
            for qt in [4, 5, 6, 7]:
                o_piece(qt)

    nc.compile()
    return nc


def _prep_inputs(x, Wq, bq, Wk, bk, Wv, bv, Wo, bo):
    bf = ml_dtypes.bfloat16
    wqT = np.ascontiguousarray(Wq.T).astype(bf)
    wkT = np.ascontiguousarray(Wk.T).astype(bf)
    wvT = np.ascontiguousarray(Wv.T).astype(bf)
    woT = np.ascontiguousarray(Wo.T).astype(bf)
    bias = np.stack([
        bq.reshape(NP, 128).T,
        bk.reshape(NP, 128).T,
    ], axis=1).astype(np.float32)          # [128, 2, NP]
    bias = np.ascontiguousarray(bias)
    bof = np.ascontiguousarray(
        (bo.astype(np.float64) + bv.astype(np.float64) @ Wo.astype(np.float64).T)
        .reshape(1, C)).astype(np.float32)
    ones1 = np.ones((1, 128), np.float32)
    in_maps = []
    for i in range(8):
        b, q0 = i // 4, (i % 4) * TQ
        xbT = np.ascontiguousarray(np.roll(x[b].T, -q0, axis=1)).astype(bf)
        in_maps.append({
            "xbT": xbT, "wqT": wqT, "wkT": wkT, "wvT": wvT, "woT": woT,
            "bias": bias, "bof": bof, "ones1": ones1,
        })
    return in_maps


def kernel(x, Wq, bq, Wk, bk, Wv, bv, Wo, bo):
    x = np.asarray(x, np.float32)
    args = [np.asarray(a, np.float32) for a in
            (Wq, bq, Wk, bk, Wv, bv, Wo, bo)]
    if "nc" not in _cache:
        _cache["nc"] = _build()
    nc = _cache["nc"]
    in_maps = _prep_inputs(x, *args)
    res = run_bass_kernel_spmd(nc, in_maps, list(range(8)))
    outf = np.empty((B, T, C), np.float32)
    for i in range(8):
        b, q0 = i // 4, (i % 4) * TQ
        outf[b, q0:q0 + TQ, :] = res.results[i]["out"]
    return outf


# revision 33
# speedup vs baseline: 1.3840x; 1.0140x over previous
"""Multi-head attention (B=2, T=4096, D=512, H=8) on 8 Trainium2 cores.

Sharding: core i handles batch b=i//4, query rows q0=(i%4)*1024 .. q0+1024,
all 8 heads (full K/V of its batch computed on-core; no collectives).
Host pre-transposes x and weights (bf16) and rolls x along T per core so
each core's query block sits at columns 0:1024.

v2 pipeline (exp-bound design):
- Projections in bf16 (x, W all bf16; f32 PSUM accumulation).
- Scores via fp8e4(e4m3) DoubleRow matmuls: K^T/Q^T drained to fp8 with a
  zeroed second k-tile slot, so one DR matmul contracts the full d_k=64 at
  0.5 cycles/row.  exp applies the 1/sqrt(d_k)=1/8 scale for free.
- exp on the Activation engine in 3-bank (1536-col) PSUM chunks, double
  buffered through a 6-bank ring; output bf16 `at` tiles.
- AV with swapped operands (stationary=at [128k,128q], moving=V|1 [128k,65])
  accumulating [128q, 65] in a single PSUM bank; per-partition rowsum ->
  reciprocal (DVE) -> normalize to bf16 (GpSimd), then DMA-transpose
  [128q,128d] -> acat [128d, q].
- bv is folded into bo on the host (bo' = bo + bv @ Wo.T), bq/bk folded
  into the fp8 drains.
- Output projection in bf16 + f32r bias matmul, f32 result.
"""
import sys
sys.path.insert(0, "/opt/trn_rl_repo")

import numpy as np
import ml_dtypes
import concourse.bacc as bacc
import concourse.mybir as mybir
import concourse.tile as tile
from concourse.bass_utils import run_bass_kernel_spmd

F32 = mybir.dt.float32
F32R = mybir.dt.float32r
BF16 = mybir.dt.bfloat16
F8 = mybir.dt.float8e4
AF = mybir.ActivationFunctionType
ADD = mybir.AluOpType.add
MULT = mybir.AluOpType.mult
DR = mybir.MatmulPerfMode.DoubleRow

B, T, C = 2, 4096, 512
H, DK = 8, 64
TQ = 1024          # queries per core
NP = 4             # head pairs
KT = T // 128      # 32 k-tiles
CT = C // 128      # 4 contraction tiles
NPH = 2 * H        # 16 phases (head, q-half)

_cache = {}


def _build():
    nc = bacc.Bacc("TRN2")
    xbT = nc.declare_dram_parameter("xbT", [C, T], BF16, isOutput=False)
    wqT = nc.declare_dram_parameter("wqT", [C, C], BF16, isOutput=False)
    wkT = nc.declare_dram_parameter("wkT", [C, C], BF16, isOutput=False)
    wvT = nc.declare_dram_parameter("wvT", [C, C], BF16, isOutput=False)
    woT = nc.declare_dram_parameter("woT", [C, C], BF16, isOutput=False)
    # bias[:, 0, p] = bq slice, bias[:, 1, p] = bk slice
    bias = nc.declare_dram_parameter("bias", [128, 2, NP], F32, isOutput=False)
    bof = nc.declare_dram_parameter("bof", [1, C], F32R, isOutput=False)
    ones1 = nc.declare_dram_parameter("ones1", [1, 128], F32R, isOutput=False)
    out = nc.declare_dram_parameter("out", [TQ, C], F32, isOutput=True)

    with tile.TileContext(nc) as tc:
        with (
            tc.tile_pool(name="big", bufs=1) as bpool,
            tc.tile_pool(name="v2", bufs=2) as v2pool,
            tc.tile_pool(name="rc", bufs=4) as rpool,
            tc.tile_pool(name="ot", bufs=4) as opool,
            tc.tile_pool(name="ring", bufs=2, space="PSUM") as ring,
            tc.tile_pool(name="avp", bufs=1, space="PSUM") as avp,
            tc.tile_pool(name="prj", bufs=1, space="PSUM") as prj,
        ):
            # ---- static SBUF tiles ----
            xT = bpool.tile([128, CT, T], BF16, tag="xT")        # 32KB/part
            woTs = bpool.tile([128, CT, C], BF16, tag="woT")     # 4KB
            biasS = bpool.tile([128, 2, NP], F32, tag="bias")
            onesO = bpool.tile([65, 128], F32R, tag="ones")
            boS = bpool.tile([65, C], F32R, tag="bo")
            # fp8 K^T/Q^T, double-buffered by pair parity; [:,1,:] stays 0
            kf8a = bpool.tile([128, 2, T], F8, tag="kf8a")
            kf8b = bpool.tile([128, 2, T], F8, tag="kf8b")
            qf8a = bpool.tile([128, 2, TQ], F8, tag="qf8a")
            qf8b = bpool.tile([128, 2, TQ], F8, tag="qf8b")
            kf8 = [kf8a, kf8b]
            qf8 = [qf8a, qf8b]
            # exp output, double-buffered by phase parity
            at0 = bpool.tile([128, KT, 512], BF16, tag="at0")    # 32KB
            at1 = bpool.tile([128, KT, 512], BF16, tag="at1")    # 32KB
            at = [at0, at1]
            # normalized AV, [q, d-pair]; double-buffered by pair parity
            avn0 = bpool.tile([128, 8, 128], BF16, tag="avn0")
            avn1 = bpool.tile([128, 8, 128], BF16, tag="avn1")
            avn = [avn0, avn1]
            acat = bpool.tile([128, NP, TQ], BF16, tag="acat")   # 8KB
            oacc = bpool.tile([128, 8, C], F32, tag="oacc")      # 16KB
            wkS = bpool.tile([128, CT, C], BF16, tag="wkS")
            wqS = bpool.tile([128, CT, C], BF16, tag="wqS")
            wvS = bpool.tile([128, CT, C], BF16, tag="wvS")

            # ---- prologue DMAs ----
            xv = xbT.rearrange("(ct p) t -> p ct t", p=128)
            wkv = wkT.rearrange("(ct p) c -> p ct c", p=128)
            wqv = wqT.rearrange("(ct p) c -> p ct c", p=128)
            wvv = wvT.rearrange("(ct p) c -> p ct c", p=128)
            nc.sync.dma_start(wkS[:], wkv[:])
            nc.sync.dma_start(xT[:, :, 0:1024], xv[:, :, 0:1024])
            nc.sync.dma_start(wqS[:], wqv[:])
            nc.sync.dma_start(biasS[:], bias[:])
            nc.sync.dma_start(wvS[:], wvv[:])
            for tch in range(1, 4):
                nc.sync.dma_start(
                    xT[:, :, tch * 1024:(tch + 1) * 1024],
                    xv[:, :, tch * 1024:(tch + 1) * 1024])
            nc.sync.dma_start(onesO[64:65, :],
                              ones1.rearrange("(o a) b -> o a b", o=1))
            nc.sync.dma_start(boS[64:65, :], bof.rearrange("(o a) b -> o a b", o=1))
            nc.sync.dma_start(woTs[:], woT.rearrange("(ct p) c -> p ct c", p=128))
            dz = bpool.tile([64, 2, 512], F8, tag="dz")
            nc.vector.memset(dz[:], 0.0)
            nc.vector.memset(kf8[0][:, 1, 0:1536], 0.0)
            nc.vector.memset(qf8[0][:, 1, :], 0.0)
            nc.gpsimd.memset(kf8[0][:, 1, 1536:T], 0.0)
            nc.gpsimd.memset(kf8[1][:, 1, :], 0.0)
            nc.gpsimd.memset(qf8[1][:, 1, :], 0.0)

            # ---- helper emitters (each returns a list of zero-arg thunks) --

            def k_piece(p, piece):
                # K^T cols piece*512:(piece+1)*512 -> kf8[p%2][:, 0, ...]
                pp = prj.tile([128, 512], F32, tag="prj")
                for ct in range(CT):
                    nc.tensor.matmul(
                        pp[:], wkS[:, ct, p * 128:(p + 1) * 128],
                        xT[:, ct, piece * 512:(piece + 1) * 512],
                        start=(ct == 0), stop=(ct == CT - 1))
                nc.vector.tensor_scalar_add(
                    kf8[p % 2][:, 0, piece * 512:(piece + 1) * 512], pp[:],
                    biasS[:, 1, p:p + 1])

            def q_piece(p, piece):
                pp = prj.tile([128, 512], F32, tag="prj")
                for ct in range(CT):
                    nc.tensor.matmul(
                        pp[:], wqS[:, ct, p * 128:(p + 1) * 128],
                        xT[:, ct, piece * 512:(piece + 1) * 512],
                        start=(ct == 0), stop=(ct == CT - 1))
                nc.vector.tensor_scalar_add(
                    qf8[p % 2][:, 0, piece * 512:(piece + 1) * 512], pp[:],
                    biasS[:, 0, p:p + 1])

            def v_piece(v2p, pg, j, pool=None):
                # V rows for k-tiles j, j+1 as separate exact-cover groups.
                for jj in range(2):
                    pv = (pool or prj).tile([128, 512], F32,
                                            tag="av" if pool is avp else "prj")
                    for ct in range(CT):
                        nc.tensor.matmul(
                            pv[:, 0:256],
                            xT[:, ct, (j + jj) * 128:(j + jj + 1) * 128],
                            wvS[:, ct, pg * 256:(pg + 1) * 256],
                            start=(ct == 0), stop=(ct == CT - 1))
                    nc.vector.tensor_copy(
                        v2p[:, j + jj, :, 0:64],
                        pv[:, 0:256].rearrange("p (h b) -> p h b", b=64))

            def new_v2p():
                v2p = v2pool.tile([128, KT, 4, 65], BF16, tag="v2p")
                nc.gpsimd.memset(v2p[:, :, :, 64], 1.0)
                return v2p

            def av_group(ph, qb, v2p, av_ap=None):
                # AV for phase ph (= head h, q-half), query block qb (0..3)
                h, half = ph // 2, ph % 2
                hb = ph % 2  # at buffer parity
                d0 = (h % 2) * 64
                pb = (h // 2) % 2
                qbg = half * 4 + qb
                if av_ap is None:
                    av_t = avp.tile([128, 512], F32, tag="av")
                else:
                    av_t = av_ap
                for kt in range(KT):
                    nc.tensor.matmul(
                        av_t[:, 0:65],
                        at[hb][:, kt, qb * 128:(qb + 1) * 128],
                        v2p[:, kt, h % 4, :],
                        start=(kt == 0), stop=(kt == KT - 1))
                rec = rpool.tile([128, 1], F32, tag="rec")
                nc.vector.reciprocal_approx_fast(rec[:], av_t[:, 64:65])
                nc.vector.tensor_scalar(
                    avn[pb][:, qbg, d0:d0 + 64], av_t[:, 0:64],
                    rec[:], None, MULT)

            def o_piece(qt, po=None):
                if po is None:
                    po = prj.tile([128, 512], F32, tag="prj")
                nc.tensor.matmul(
                    po[:], acat[:, 3, qt * 128:(qt + 1) * 128],
                    woTs[:, 3, :], start=True, stop=True)
                ot = opool.tile([128, 512], F32, tag="ot")
                nc.vector.tensor_tensor(out=ot[:], in0=po[:],
                                        in1=oacc[:, qt, :], op=ADD)
                nc.sync.dma_start(out[qt * 128:(qt + 1) * 128, :], ot[:])

            # ---- prologue compute: pair-0 K/Q on ring slots (batched drains) --

            def ring_kq(groups):
                """groups: list of ('k'|'q', p, piece). One ring tile, one
                matmul group per bank, batched drains per contiguous run."""
                rt = ring.tile([128, 1536], F32, tag="ring")
                for g, (kind, p, piece) in enumerate(groups):
                    w = wkS if kind == "k" else wqS
                    for ct in range(CT):
                        nc.tensor.matmul(
                            rt[:, g * 512:(g + 1) * 512],
                            w[:, ct, p * 128:(p + 1) * 128],
                            xT[:, ct, piece * 512:(piece + 1) * 512],
                            start=(ct == 0), stop=(ct == CT - 1))
                # batched drains over contiguous same-kind runs
                g = 0
                while g < len(groups):
                    kind, p, piece = groups[g]
                    g2 = g
                    while (g2 + 1 < len(groups)
                           and groups[g2 + 1][0] == kind
                           and groups[g2 + 1][2] == groups[g2][2] + 1):
                        g2 += 1
                    dst = kf8[p % 2] if kind == "k" else qf8[p % 2]
                    bcol = 1 if kind == "k" else 0
                    nc.vector.tensor_scalar_add(
                        dst[:, 0, piece * 512:piece * 512 + (g2 - g + 1) * 512],
                        rt[:, g * 512:(g2 + 1) * 512],
                        biasS[:, bcol, p:p + 1])
                    g = g2 + 1

            # PE p-state warm-up on zeros while x loads
            wup = avp.tile([128, 512], F32, tag="av")
            for i in range(14):
                nc.tensor.matmul(wup[:], dz[:, :, 0:128], dz[:],
                                 start=True, stop=True, perf_mode=DR,
                                 tile_position=(0, 0))
            ring_kq([("k", 0, 0)])
            q_piece(0, 0)
            ring_kq([("q", 0, 1), ("k", 0, 1), ("k", 0, 2)])
            ring_kq([("k", 0, 3), ("k", 0, 4), ("k", 0, 5)])
            v2p_cur = new_v2p()

            # ---- main pipeline over 16 phases ----
            state = {"v2p": v2p_cur, "v2p_next": None, "pending": [],
                     "o_pending": []}

            def phase_background(ph):
                """Thunks to interleave into phase ph's chunk stream."""
                thunks = []
                h, half = ph // 2, ph % 2
                p = h // 2
                # deferred transposes/O-pieces from the previous phase first
                pend, state["pending"] = state["pending"], []
                thunks.extend(pend)
                # leftover pg0 V-pieces MUST precede phase-0's AV groups
                if ph == 1:
                    for j in range(28, KT, 2):
                        thunks.append(lambda j=j: v_piece(state["v2p"], 0, j))
                # AV of previous phase (+ deferred transpose & O-proj)
                if ph >= 1:
                    prev_h = (ph - 1) // 2
                    v2p_prev = (state["v2p_prev4"] if prev_h // 4 != h // 4
                                else state["v2p"])
                    for qb in range(4):
                        thunks.append(lambda ph=ph, qb=qb, v=v2p_prev:
                                      av_tr_o(ph - 1, qb, v))
                # projection prep for pair p+1
                slot = ph % 4
                if p + 1 < NP:
                    if slot == 2:
                        for piece in range(5):
                            thunks.append(lambda p=p, piece=piece:
                                          k_piece(p + 1, piece))
                    elif slot == 3:
                        for piece in range(5, 8):
                            thunks.append(lambda p=p, piece=piece:
                                          k_piece(p + 1, piece))
                        for piece in range(2):
                            thunks.append(lambda p=p, piece=piece:
                                          q_piece(p + 1, piece))
                # pair-0 K piece 7 early in phase 0
                if ph == 0:
                    thunks.append(lambda: k_piece(0, 6))
                    thunks.append(lambda: k_piece(0, 7))
                    for i, j in enumerate(range(0, 28, 2)):
                        thunks.append(lambda j=j, i=i: v_piece(
                            state["v2p"], 0, j,
                            pool=avp if i % 2 else prj))
                if 4 <= ph <= 7:
                    if ph == 4:
                        def mkv():
                            state["v2p_next"] = new_v2p()
                        thunks.append(mkv)
                    for j in range((ph - 4) * 8, (ph - 4) * 8 + 8, 2):
                        thunks.append(lambda j=j: v_piece(state["v2p_next"],
                                                          1, j))
                if ph == 13:
                    for qt in range(4):
                        thunks.append(lambda qt=qt: o_partial(qt))
                if ph == 14:
                    for qt in range(4, 8):
                        thunks.append(lambda qt=qt: o_partial(qt))
                return thunks

            def o_partial(qt):
                po = prj.tile([128, 512], F32, tag="prj")
                for r in range(3):
                    nc.tensor.matmul(
                        po[:], acat[:, r, qt * 128:(qt + 1) * 128],
                        woTs[:, r, :], start=(r == 0), stop=False)
                nc.tensor.matmul(po[:], onesO[64:65, :], boS[64:65, :],
                                 start=False, stop=True)
                nc.vector.tensor_copy(oacc[:, qt, :], po[:])

            def tr_o(p, qbg, po=None):
                nc.sync.dma_start_transpose(
                    acat[:, p, qbg * 128:(qbg + 1) * 128],
                    avn[p % 2][:, qbg, :])
                if p == NP - 1:
                    o_piece(qbg, po)

            def av_tr_o(ph, qb, v2p, av_ap=None, po=None, defer=True):
                """AV group + (for odd heads) transpose + (pair 3) O-proj."""
                av_group(ph, qb, v2p, av_ap)
                h, half = ph // 2, ph % 2
                if h % 2 == 1:
                    p = h // 2
                    qbg = half * 4 + qb
                    if p == NP - 1:
                        # last pair: transpose deferred, O-piece to epilogue
                        state["pending"].append(
                            lambda qbg=qbg: nc.sync.dma_start_transpose(
                                acat[:, 3, qbg * 128:(qbg + 1) * 128],
                                avn[1][:, qbg, :]))
                        state["o_pending"].append(qbg)
                    elif defer:
                        state["pending"].append(
                            lambda p=p, qbg=qbg, po=po: tr_o(p, qbg, po))
                    else:
                        tr_o(p, qbg, po)

            for ph in range(NPH):
                h, half = ph // 2, ph % 2
                if ph == 8:
                    state["v2p_prev4"] = state["v2p"]
                    state["v2p"] = state["v2p_next"]
                state.setdefault("v2p_prev4", state["v2p"])
                d0 = (h % 2) * 64
                kcur, qcur = kf8[h // 2 % 2], qf8[h // 2 % 2]
                bg = phase_background(ph)
                bgi = 0
                # 11 chunks: 10x3 kt + 1x2 kt
                for c in range(11):
                    n = 3 if c < 10 else 2
                    ring_t = ring.tile([128, 1536], F32, tag="ring")
                    for jj in range(n):
                        kt = 3 * c + jj
                        nc.tensor.matmul(
                            ring_t[:, jj * 512:(jj + 1) * 512],
                            kcur[d0:d0 + 64, :, kt * 128:(kt + 1) * 128],
                            qcur[d0:d0 + 64, :, half * 512:(half + 1) * 512],
                            start=True, stop=True, perf_mode=DR,
                            tile_position=(d0, 0))
                    nc.scalar.activation(
                        at[ph % 2][:, 3 * c:3 * c + n, :],
                        ring_t[:, 0:512 * n].rearrange("p (a b) -> p a b", b=512),
                        AF.Exp, scale=0.125)
                    # interleave background work: spread across chunks
                    n_bg = (len(bg) * (c + 1)) // 11 - (len(bg) * c) // 11
                    for _ in range(n_bg):
                        bg[bgi]()
                        bgi += 1
                assert bgi == len(bg)
                if ph == NPH - 1:
                    # flush pair-3 transposes first, then their O-pieces
                    pend, state["pending"] = state["pending"], []
                    for th in pend:
                        th()
                    for qt in state["o_pending"][:4]:
                        o_piece(qt)
                    state["o_pending"] = state["o_pending"][4:]

            # ---- epilogue: AV of phase 15 + pair-3 transposes + O-proj ----
            # ring banks are free: give every AV group and O-piece its own
            # bank and software-pipeline so no PE wait blocks later work.
            for th in state["pending"]:
                th()
            h15 = (NPH - 1) // 2
            v2p15 = state["v2p"]
            avA = avp.tile([128, 512], F32, tag="av")
            epA = ring.tile([128, 1536], F32, tag="ring")
            av_aps = [avA, epA[:, 0:512], epA[:, 512:1024], epA[:, 1024:1536]]
            for kt in range(KT):
                for qb in range(4):
                    nc.tensor.matmul(
                        av_aps[qb][:, 0:65],
                        at[(NPH - 1) % 2][:, kt, qb * 128:(qb + 1) * 128],
                        v2p15[:, kt, h15 % 4, :],
                        start=(kt == 0), stop=(kt == KT - 1))

            def norm_tr(qb, t):
                qbg = 4 + qb
                rec = rpool.tile([128, 1], F32, tag="rec")
                nc.vector.reciprocal_approx_fast(rec[:], t[:, 64:65])
                nc.vector.tensor_scalar(
                    avn[1][:, qbg, 64:128], t[:, 0:64], rec[:], None, MULT)
                nc.sync.dma_start_transpose(
                    acat[:, 3, qbg * 128:(qbg + 1) * 128],
                    avn[1][:, qbg, :])

            for qb in range(4):
                norm_tr(qb, av_aps[qb])
            epB = ring.tile([128, 1536], F32, tag="ring")
            avB = avp.tile([128, 512], F32, tag="av")
            for i, qt in enumerate([4, 5, 6, 7]):
                o_piece(qt, po=epB[:, i * 512:(i + 1) * 512]
                        if i < 3 else avB)

    nc.compile()
    return nc


def _prep_inputs(x, Wq, bq, Wk, bk, Wv, bv, Wo, bo):
    bf = ml_dtypes.bfloat16
    wqT = np.ascontiguousarray(Wq.T).astype(bf)
    wkT = np.ascontiguousarray(Wk.T).astype(bf)
    wvT = np.ascontiguousarray(Wv.T).astype(bf)
    woT = np.ascontiguousarray(Wo.T).astype(bf)
    bias = np.stack([
        bq.reshape(NP, 128).T,
        bk.reshape(NP, 128).T,
    ], axis=1).astype(np.float32)          # [128, 2, NP]
    bias = np.ascontiguousarray(bias)
    bof = np.ascontiguousarray(
        (bo.astype(np.float64) + bv.astype(np.float64) @ Wo.astype(np.float64).T)
        .reshape(1, C)).astype(np.float32)
    ones1 = np.ones((1, 128), np.float32)
    in_maps = []
    for i in range(8):
        b, q0 = i // 4, (i % 4) * TQ
        xbT = np.ascontiguousarray(np.roll(x[b].T, -q0, axis=1)).astype(bf)
        in_maps.append({
            "xbT": xbT, "wqT": wqT, "wkT": wkT, "wvT": wvT, "woT": woT,
            "bias": bias, "bof": bof, "ones1": ones1,
        })
    return in_maps


def kernel(x, Wq, bq, Wk, bk, Wv, bv, Wo, bo):
    x = np.asarray(x, np.float32)
    args = [np.asarray(a, np.float32) for a in
            (Wq, bq, Wk, bk, Wv, bv, Wo, bo)]
    if "nc" not in _cache:
        _cache["nc"] = _build()
    nc = _cache["nc"]
    in_maps = _prep_inputs(x, *args)
    res = run_bass_kernel_spmd(nc, in_maps, list(range(8)))
    outf = np.empty((B, T, C), np.float32)
    for i in range(8):
        b, q0 = i // 4, (i % 4) * TQ
        outf[b, q0:q0 + TQ, :] = res.results[i]["out"]
    return outf


# revision 34
# speedup vs baseline: 1.3960x; 1.0087x over previous
"""Multi-head attention (B=2, T=4096, D=512, H=8) on 8 Trainium2 cores.

Sharding: core i handles batch b=i//4, query rows q0=(i%4)*1024 .. q0+1024,
all 8 heads (full K/V of its batch computed on-core; no collectives).
Host pre-transposes x and weights (bf16) and rolls x along T per core so
each core's query block sits at columns 0:1024.

v2 pipeline (exp-bound design):
- Projections in bf16 (x, W all bf16; f32 PSUM accumulation).
- Scores via fp8e4(e4m3) DoubleRow matmuls: K^T/Q^T drained to fp8 with a
  zeroed second k-tile slot, so one DR matmul contracts the full d_k=64 at
  0.5 cycles/row.  exp applies the 1/sqrt(d_k)=1/8 scale for free.
- exp on the Activation engine in 3-bank (1536-col) PSUM chunks, double
  buffered through a 6-bank ring; output bf16 `at` tiles.
- AV with swapped operands (stationary=at [128k,128q], moving=V|1 [128k,65])
  accumulating [128q, 65] in a single PSUM bank; per-partition rowsum ->
  reciprocal (DVE) -> normalize to bf16 (GpSimd), then DMA-transpose
  [128q,128d] -> acat [128d, q].
- bv is folded into bo on the host (bo' = bo + bv @ Wo.T), bq/bk folded
  into the fp8 drains.
- Output projection in bf16 + f32r bias matmul, f32 result.
"""
import sys
sys.path.insert(0, "/opt/trn_rl_repo")

import numpy as np
import ml_dtypes
import concourse.bacc as bacc
import concourse.mybir as mybir
import concourse.tile as tile
from concourse.bass_utils import run_bass_kernel_spmd

F32 = mybir.dt.float32
F32R = mybir.dt.float32r
BF16 = mybir.dt.bfloat16
F8 = mybir.dt.float8e4
AF = mybir.ActivationFunctionType
ADD = mybir.AluOpType.add
MULT = mybir.AluOpType.mult
DR = mybir.MatmulPerfMode.DoubleRow

B, T, C = 2, 4096, 512
H, DK = 8, 64
TQ = 1024          # queries per core
NP = 4             # head pairs
KT = T // 128      # 32 k-tiles
CT = C // 128      # 4 contraction tiles
NPH = 2 * H        # 16 phases (head, q-half)

_cache = {}


def _build():
    nc = bacc.Bacc("TRN2")
    xbT = nc.declare_dram_parameter("xbT", [C, T], BF16, isOutput=False)
    wqT = nc.declare_dram_parameter("wqT", [C, C], BF16, isOutput=False)
    wkT = nc.declare_dram_parameter("wkT", [C, C], BF16, isOutput=False)
    wvT = nc.declare_dram_parameter("wvT", [C, C], BF16, isOutput=False)
    woT = nc.declare_dram_parameter("woT", [C, C], BF16, isOutput=False)
    # bias[:, 0, p] = bq slice, bias[:, 1, p] = bk slice
    bias = nc.declare_dram_parameter("bias", [128, 2, NP], F32, isOutput=False)
    bof = nc.declare_dram_parameter("bof", [1, C], F32R, isOutput=False)
    ones1 = nc.declare_dram_parameter("ones1", [1, 128], F32R, isOutput=False)
    out = nc.declare_dram_parameter("out", [TQ, C], F32, isOutput=True)

    with tile.TileContext(nc) as tc:
        with (
            tc.tile_pool(name="big", bufs=1) as bpool,
            tc.tile_pool(name="v2", bufs=2) as v2pool,
            tc.tile_pool(name="rc", bufs=4) as rpool,
            tc.tile_pool(name="ot", bufs=4) as opool,
            tc.tile_pool(name="ring", bufs=2, space="PSUM") as ring,
            tc.tile_pool(name="avp", bufs=1, space="PSUM") as avp,
            tc.tile_pool(name="prj", bufs=1, space="PSUM") as prj,
        ):
            # ---- static SBUF tiles ----
            xT = bpool.tile([128, CT, T], BF16, tag="xT")        # 32KB/part
            woTs = bpool.tile([128, CT, C], BF16, tag="woT")     # 4KB
            biasS = bpool.tile([128, 2, NP], F32, tag="bias")
            onesO = bpool.tile([65, 128], F32R, tag="ones")
            boS = bpool.tile([65, C], F32R, tag="bo")
            # fp8 K^T/Q^T, double-buffered by pair parity; [:,1,:] stays 0
            kf8a = bpool.tile([128, 2, T], F8, tag="kf8a")
            kf8b = bpool.tile([128, 2, T], F8, tag="kf8b")
            qf8a = bpool.tile([128, 2, TQ], F8, tag="qf8a")
            qf8b = bpool.tile([128, 2, TQ], F8, tag="qf8b")
            kf8 = [kf8a, kf8b]
            qf8 = [qf8a, qf8b]
            # exp output, double-buffered by phase parity
            at0 = bpool.tile([128, KT, 512], BF16, tag="at0")    # 32KB
            at1 = bpool.tile([128, KT, 512], BF16, tag="at1")    # 32KB
            at = [at0, at1]
            # normalized AV, [q, d-pair]; double-buffered by pair parity
            avn0 = bpool.tile([128, 8, 128], BF16, tag="avn0")
            avn1 = bpool.tile([128, 8, 128], BF16, tag="avn1")
            avn = [avn0, avn1]
            acat = bpool.tile([128, NP, TQ], BF16, tag="acat")   # 8KB
            oacc = bpool.tile([128, 8, C], F32, tag="oacc")      # 16KB
            wkS = bpool.tile([128, CT, C], BF16, tag="wkS")
            wqS = bpool.tile([128, CT, C], BF16, tag="wqS")
            wvS = bpool.tile([128, CT, C], BF16, tag="wvS")

            # ---- prologue DMAs ----
            xv = xbT.rearrange("(ct p) t -> p ct t", p=128)
            wkv = wkT.rearrange("(ct p) c -> p ct c", p=128)
            wqv = wqT.rearrange("(ct p) c -> p ct c", p=128)
            wvv = wvT.rearrange("(ct p) c -> p ct c", p=128)
            nc.sync.dma_start(wkS[:], wkv[:])
            nc.sync.dma_start(xT[:, :, 0:512], xv[:, :, 0:512])
            nc.sync.dma_start(wqS[:], wqv[:])
            nc.sync.dma_start(biasS[:], bias[:])
            nc.sync.dma_start(xT[:, :, 512:1024], xv[:, :, 512:1024])
            nc.sync.dma_start(wvS[:], wvv[:])
            for tch in range(1, 4):
                nc.sync.dma_start(
                    xT[:, :, tch * 1024:(tch + 1) * 1024],
                    xv[:, :, tch * 1024:(tch + 1) * 1024])
            nc.sync.dma_start(onesO[64:65, :],
                              ones1.rearrange("(o a) b -> o a b", o=1))
            nc.sync.dma_start(boS[64:65, :], bof.rearrange("(o a) b -> o a b", o=1))
            nc.sync.dma_start(woTs[:], woT.rearrange("(ct p) c -> p ct c", p=128))
            dz = bpool.tile([64, 2, 512], F8, tag="dz")
            nc.vector.memset(dz[:], 0.0)
            nc.vector.memset(kf8[0][:, 1, 0:1536], 0.0)
            nc.vector.memset(qf8[0][:, 1, :], 0.0)
            nc.gpsimd.memset(kf8[0][:, 1, 1536:T], 0.0)
            nc.gpsimd.memset(kf8[1][:, 1, :], 0.0)
            nc.gpsimd.memset(qf8[1][:, 1, :], 0.0)

            # ---- helper emitters (each returns a list of zero-arg thunks) --

            def k_piece(p, piece):
                # K^T cols piece*512:(piece+1)*512 -> kf8[p%2][:, 0, ...]
                pp = prj.tile([128, 512], F32, tag="prj")
                for ct in range(CT):
                    nc.tensor.matmul(
                        pp[:], wkS[:, ct, p * 128:(p + 1) * 128],
                        xT[:, ct, piece * 512:(piece + 1) * 512],
                        start=(ct == 0), stop=(ct == CT - 1))
                nc.vector.tensor_scalar_add(
                    kf8[p % 2][:, 0, piece * 512:(piece + 1) * 512], pp[:],
                    biasS[:, 1, p:p + 1])

            def q_piece(p, piece):
                pp = prj.tile([128, 512], F32, tag="prj")
                for ct in range(CT):
                    nc.tensor.matmul(
                        pp[:], wqS[:, ct, p * 128:(p + 1) * 128],
                        xT[:, ct, piece * 512:(piece + 1) * 512],
                        start=(ct == 0), stop=(ct == CT - 1))
                nc.vector.tensor_scalar_add(
                    qf8[p % 2][:, 0, piece * 512:(piece + 1) * 512], pp[:],
                    biasS[:, 0, p:p + 1])

            def v_piece(v2p, pg, j, pool=None):
                # V rows for k-tiles j, j+1 as separate exact-cover groups.
                for jj in range(2):
                    pv = (pool or prj).tile([128, 512], F32,
                                            tag="av" if pool is avp else "prj")
                    for ct in range(CT):
                        nc.tensor.matmul(
                            pv[:, 0:256],
                            xT[:, ct, (j + jj) * 128:(j + jj + 1) * 128],
                            wvS[:, ct, pg * 256:(pg + 1) * 256],
                            start=(ct == 0), stop=(ct == CT - 1))
                    nc.vector.tensor_copy(
                        v2p[:, j + jj, :, 0:64],
                        pv[:, 0:256].rearrange("p (h b) -> p h b", b=64))

            def new_v2p():
                v2p = v2pool.tile([128, KT, 4, 65], BF16, tag="v2p")
                nc.gpsimd.memset(v2p[:, :, :, 64], 1.0)
                return v2p

            def av_group(ph, qb, v2p, av_ap=None):
                # AV for phase ph (= head h, q-half), query block qb (0..3)
                h, half = ph // 2, ph % 2
                hb = ph % 2  # at buffer parity
                d0 = (h % 2) * 64
                pb = (h // 2) % 2
                qbg = half * 4 + qb
                if av_ap is None:
                    av_t = avp.tile([128, 512], F32, tag="av")
                else:
                    av_t = av_ap
                for kt in range(KT):
                    nc.tensor.matmul(
                        av_t[:, 0:65],
                        at[hb][:, kt, qb * 128:(qb + 1) * 128],
                        v2p[:, kt, h % 4, :],
                        start=(kt == 0), stop=(kt == KT - 1))
                rec = rpool.tile([128, 1], F32, tag="rec")
                nc.vector.reciprocal_approx_fast(rec[:], av_t[:, 64:65])
                nc.vector.tensor_scalar(
                    avn[pb][:, qbg, d0:d0 + 64], av_t[:, 0:64],
                    rec[:], None, MULT)

            def o_piece(qt, po=None):
                if po is None:
                    po = prj.tile([128, 512], F32, tag="prj")
                nc.tensor.matmul(
                    po[:], acat[:, 3, qt * 128:(qt + 1) * 128],
                    woTs[:, 3, :], start=True, stop=True)
                ot = opool.tile([128, 512], F32, tag="ot")
                nc.vector.tensor_tensor(out=ot[:], in0=po[:],
                                        in1=oacc[:, qt, :], op=ADD)
                nc.sync.dma_start(out[qt * 128:(qt + 1) * 128, :], ot[:])

            # ---- prologue compute: pair-0 K/Q on ring slots (batched drains) --

            def ring_kq(groups):
                """groups: list of ('k'|'q', p, piece). One ring tile, one
                matmul group per bank, batched drains per contiguous run."""
                rt = ring.tile([128, 1536], F32, tag="ring")
                for g, (kind, p, piece) in enumerate(groups):
                    w = wkS if kind == "k" else wqS
                    for ct in range(CT):
                        nc.tensor.matmul(
                            rt[:, g * 512:(g + 1) * 512],
                            w[:, ct, p * 128:(p + 1) * 128],
                            xT[:, ct, piece * 512:(piece + 1) * 512],
                            start=(ct == 0), stop=(ct == CT - 1))
                # batched drains over contiguous same-kind runs
                g = 0
                while g < len(groups):
                    kind, p, piece = groups[g]
                    g2 = g
                    while (g2 + 1 < len(groups)
                           and groups[g2 + 1][0] == kind
                           and groups[g2 + 1][2] == groups[g2][2] + 1):
                        g2 += 1
                    dst = kf8[p % 2] if kind == "k" else qf8[p % 2]
                    bcol = 1 if kind == "k" else 0
                    nc.vector.tensor_scalar_add(
                        dst[:, 0, piece * 512:piece * 512 + (g2 - g + 1) * 512],
                        rt[:, g * 512:(g2 + 1) * 512],
                        biasS[:, bcol, p:p + 1])
                    g = g2 + 1

            # PE p-state warm-up on zeros while x loads
            wup = avp.tile([128, 512], F32, tag="av")
            for i in range(14):
                nc.tensor.matmul(wup[:], dz[:, :, 0:128], dz[:],
                                 start=True, stop=True, perf_mode=DR,
                                 tile_position=(0, 0))
            ring_kq([("k", 0, 0)])
            q_piece(0, 0)
            v2p_cur = new_v2p()

            # ---- main pipeline over 16 phases ----
            state = {"v2p": v2p_cur, "v2p_next": None, "pending": [],
                     "o_pending": []}

            def phase_background(ph):
                """Thunks to interleave into phase ph's chunk stream."""
                thunks = []
                h, half = ph // 2, ph % 2
                p = h // 2
                # deferred transposes/O-pieces from the previous phase first
                pend, state["pending"] = state["pending"], []
                thunks.extend(pend)
                # leftover pg0 V-pieces MUST precede phase-0's AV groups
                if ph == 1:
                    for j in range(28, KT, 2):
                        thunks.append(lambda j=j: v_piece(state["v2p"], 0, j))
                # AV of previous phase (+ deferred transpose & O-proj)
                if ph >= 1:
                    prev_h = (ph - 1) // 2
                    v2p_prev = (state["v2p_prev4"] if prev_h // 4 != h // 4
                                else state["v2p"])
                    for qb in range(4):
                        thunks.append(lambda ph=ph, qb=qb, v=v2p_prev:
                                      av_tr_o(ph - 1, qb, v))
                # projection prep for pair p+1
                slot = ph % 4
                if p + 1 < NP:
                    if slot == 2:
                        for piece in range(5):
                            thunks.append(lambda p=p, piece=piece:
                                          k_piece(p + 1, piece))
                    elif slot == 3:
                        for piece in range(5, 8):
                            thunks.append(lambda p=p, piece=piece:
                                          k_piece(p + 1, piece))
                        for piece in range(2):
                            thunks.append(lambda p=p, piece=piece:
                                          q_piece(p + 1, piece))
                # pair-0 K piece 7 early in phase 0
                if ph == 0:
                    thunks.append(lambda: ring_kq(
                        [("q", 0, 1), ("k", 0, 1), ("k", 0, 2)]))
                    thunks.append(lambda: ring_kq(
                        [("k", 0, 3), ("k", 0, 4), ("k", 0, 5)]))
                    thunks.append(lambda: k_piece(0, 6))
                    thunks.append(lambda: k_piece(0, 7))
                    for i, j in enumerate(range(0, 28, 2)):
                        thunks.append(lambda j=j, i=i: v_piece(
                            state["v2p"], 0, j,
                            pool=avp if i % 2 else prj))
                if 4 <= ph <= 7:
                    if ph == 4:
                        def mkv():
                            state["v2p_next"] = new_v2p()
                        thunks.append(mkv)
                    for j in range((ph - 4) * 8, (ph - 4) * 8 + 8, 2):
                        thunks.append(lambda j=j: v_piece(state["v2p_next"],
                                                          1, j))
                if ph == 13:
                    for qt in range(4):
                        thunks.append(lambda qt=qt: o_partial(qt))
                if ph == 14:
                    for qt in range(4, 8):
                        thunks.append(lambda qt=qt: o_partial(qt))
                return thunks

            def o_partial(qt):
                po = prj.tile([128, 512], F32, tag="prj")
                for r in range(3):
                    nc.tensor.matmul(
                        po[:], acat[:, r, qt * 128:(qt + 1) * 128],
                        woTs[:, r, :], start=(r == 0), stop=False)
                nc.tensor.matmul(po[:], onesO[64:65, :], boS[64:65, :],
                                 start=False, stop=True)
                nc.vector.tensor_copy(oacc[:, qt, :], po[:])

            def tr_o(p, qbg, po=None):
                nc.sync.dma_start_transpose(
                    acat[:, p, qbg * 128:(qbg + 1) * 128],
                    avn[p % 2][:, qbg, :])
                if p == NP - 1:
                    o_piece(qbg, po)

            def av_tr_o(ph, qb, v2p, av_ap=None, po=None, defer=True):
                """AV group + (for odd heads) transpose + (pair 3) O-proj."""
                av_group(ph, qb, v2p, av_ap)
                h, half = ph // 2, ph % 2
                if h % 2 == 1:
                    p = h // 2
                    qbg = half * 4 + qb
                    if p == NP - 1:
                        # last pair: transpose deferred, O-piece to epilogue
                        state["pending"].append(
                            lambda qbg=qbg: nc.sync.dma_start_transpose(
                                acat[:, 3, qbg * 128:(qbg + 1) * 128],
                                avn[1][:, qbg, :]))
                        state["o_pending"].append(qbg)
                    elif defer:
                        state["pending"].append(
                            lambda p=p, qbg=qbg, po=po: tr_o(p, qbg, po))
                    else:
                        tr_o(p, qbg, po)

            for ph in range(NPH):
                h, half = ph // 2, ph % 2
                if ph == 8:
                    state["v2p_prev4"] = state["v2p"]
                    state["v2p"] = state["v2p_next"]
                state.setdefault("v2p_prev4", state["v2p"])
                d0 = (h % 2) * 64
                kcur, qcur = kf8[h // 2 % 2], qf8[h // 2 % 2]
                bg = phase_background(ph)
                bgi = 0
                # 11 chunks: 10x3 kt + 1x2 kt
                for c in range(11):
                    n = 3 if c < 10 else 2
                    ring_t = ring.tile([128, 1536], F32, tag="ring")
                    for jj in range(n):
                        kt = 3 * c + jj
                        nc.tensor.matmul(
                            ring_t[:, jj * 512:(jj + 1) * 512],
                            kcur[d0:d0 + 64, :, kt * 128:(kt + 1) * 128],
                            qcur[d0:d0 + 64, :, half * 512:(half + 1) * 512],
                            start=True, stop=True, perf_mode=DR,
                            tile_position=(d0, 0))
                    nc.scalar.activation(
                        at[ph % 2][:, 3 * c:3 * c + n, :],
                        ring_t[:, 0:512 * n].rearrange("p (a b) -> p a b", b=512),
                        AF.Exp, scale=0.125)
                    # interleave background work: spread across chunks
                    n_bg = (len(bg) * (c + 1)) // 11 - (len(bg) * c) // 11
                    for _ in range(n_bg):
                        bg[bgi]()
                        bgi += 1
                assert bgi == len(bg)
                if ph == NPH - 1:
                    # flush pair-3 transposes first, then their O-pieces
                    pend, state["pending"] = state["pending"], []
                    for th in pend:
                        th()
                    for qt in state["o_pending"][:4]:
                        o_piece(qt)
                    state["o_pending"] = state["o_pending"][4:]

            # ---- epilogue: AV of phase 15 + pair-3 transposes + O-proj ----
            # ring banks are free: give every AV group and O-piece its own
            # bank and software-pipeline so no PE wait blocks later work.
            for th in state["pending"]:
                th()
            h15 = (NPH - 1) // 2
            v2p15 = state["v2p"]
            avA = avp.tile([128, 512], F32, tag="av")
            epA = ring.tile([128, 1536], F32, tag="ring")
            av_aps = [avA, epA[:, 0:512], epA[:, 512:1024], epA[:, 1024:1536]]
            for kt in range(KT):
                for qb in range(4):
                    nc.tensor.matmul(
                        av_aps[qb][:, 0:65],
                        at[(NPH - 1) % 2][:, kt, qb * 128:(qb + 1) * 128],
                        v2p15[:, kt, h15 % 4, :],
                        start=(kt == 0), stop=(kt == KT - 1))

            def norm_tr(qb, t):
                qbg = 4 + qb
                rec = rpool.tile([128, 1], F32, tag="rec")
                nc.vector.reciprocal_approx_fast(rec[:], t[:, 64:65])
                nc.vector.tensor_scalar(
                    avn[1][:, qbg, 64:128], t[:, 0:64], rec[:], None, MULT)
                nc.sync.dma_start_transpose(
                    acat[:, 3, qbg * 128:(qbg + 1) * 128],
                    avn[1][:, qbg, :])

            for qb in range(4):
                norm_tr(qb, av_aps[qb])
            epB = ring.tile([128, 1536], F32, tag="ring")
            avB = avp.tile([128, 512], F32, tag="av")
            for i, qt in enumerate([4, 5, 6, 7]):
                o_piece(qt, po=epB[:, i * 512:(i + 1) * 512]
                        if i < 3 else avB)

    nc.compile()
    return nc


def _prep_inputs(x, Wq, bq, Wk, bk, Wv, bv, Wo, bo):
    bf = ml_dtypes.bfloat16
    wqT = np.ascontiguousarray(Wq.T).astype(bf)
    wkT = np.ascontiguousarray(Wk.T).astype(bf)
    wvT = np.ascontiguousarray(Wv.T).astype(bf)
    woT = np.ascontiguousarray(Wo.T).astype(bf)
    bias = np.stack([
        bq.reshape(NP, 128).T,
        bk.reshape(NP, 128).T,
    ], axis=1).astype(np.float32)          # [128, 2, NP]
    bias = np.ascontiguousarray(bias)
    bof = np.ascontiguousarray(
        (bo.astype(np.float64) + bv.astype(np.float64) @ Wo.astype(np.float64).T)
        .reshape(1, C)).astype(np.float32)
    ones1 = np.ones((1, 128), np.float32)
    in_maps = []
    for i in range(8):
        b, q0 = i // 4, (i % 4) * TQ
        xbT = np.ascontiguousarray(np.roll(x[b].T, -q0, axis=1)).astype(bf)
        in_maps.append({
            "xbT": xbT, "wqT": wqT, "wkT": wkT, "wvT": wvT, "woT": woT,
            "bias": bias, "bof": bof, "ones1": ones1,
        })
    return in_maps


def kernel(x, Wq, bq, Wk, bk, Wv, bv, Wo, bo):
    x = np.asarray(x, np.float32)
    args = [np.asarray(a, np.float32) for a in
            (Wq, bq, Wk, bk, Wv, bv, Wo, bo)]
    if "nc" not in _cache:
        _cache["nc"] = _build()
    nc = _cache["nc"]
    in_maps = _prep_inputs(x, *args)
    res = run_bass_kernel_spmd(nc, in_maps, list(range(8)))
    outf = np.empty((B, T, C), np.float32)
    for i in range(8):
        b, q0 = i // 4, (i % 4) * TQ
        outf[b, q0:q0 + TQ, :] = res.results[i]["out"]
    return outf
